# revision 8
# baseline (speedup 1.0000x reference)
"""Bass/Trainium2 kernel for nn_Attention_481036337480.

Sharding: 8 cores = 4 batches x 2 query-halves. Core c handles batch
b=c//2 and query rows l' in [1024g, 1024(g+1)) (g=c%2) for ALL 8 heads.
The reference's raw reshape (B,L,D)->(B,H,L,dh) means head h only
involves projection rows [256h, 256h+256), so every per-core output
slice (attn_dist rows, out rows) is disjoint -> no collectives.

Layout trick: the reshape interleave l' = 8*r + c means head-h q/k/v
transposed slices are NOT plain APs of proj^T. Instead of de-interleaving
(which would need cross-partition copies the engines can't do), scores
are computed directly from projT psum layout as (cq, ck) 64-contraction
sub-blocks; a host-rolled copy of Wq supplies lhsT at the opposite
partition base for parity-mismatched (cq, ck) pairs. The blocked k-order
is un-permuted for free inside the strided-output exp pass.

Softmax: additive mask bias (host: (m-1)*1e9, k-block-permuted cols),
exp on ScalarE with accum_out denominators, normalize on GPSIMD.
"""

import sys

sys.path.insert(0, "/opt/trn_rl_repo")

import numpy as np

B, L, D, H = 4, 2048, 512, 8
DH = D // H  # 64
LQ = 1024  # q rows per core
NCORES = 8
CKS = [0, 2, 4, 6, 1, 3, 5, 7]  # k-block order as produced in scores psum

_CACHE = {}


def _patch_tile_drain():
    """walrus CTRL instructions accept only 1 sem wait; Tile's final drain
    attaches one wait per still-open semaphore. Chunk across chained drains."""
    from concourse.tile import TileContext
    from concourse.vector_clock import ScopedClock
    from concourse import mybir

    if getattr(TileContext, "_drain_patched", False):
        return

    def _drain_and_barrier(self, tick_clock, wait_clock):
        nc = self.nc
        drain_inst = nc.sync.drain()
        wait_clock.add_sem_waits(
            drain_inst.ins, ScopedClock({None: tick_clock.global_clock})
        )
        si = drain_inst.ins.sync_info
        waits = list(si.on_wait or [])
        if len(waits) > 1:
            si.on_wait.clear()
            si.on_wait.append(waits[0])
            for w in waits[1:]:
                extra = nc.sync.drain()
                if extra.ins.sync_info is None:
                    extra.ins.sync_info = mybir.SyncInfo(on_wait=[], on_update=[])
                extra.ins.sync_info.on_wait.append(w)
        nc.all_engine_barrier()
        assert self.sems is not None
        popped = nc._tile_sem_poison_stack.pop()
        assert popped is self._sem_poison
        nc.clear_and_free_semaphores(list(self.sems.allocated().values()))
        nc.all_engine_barrier()

    TileContext._drain_and_barrier = _drain_and_barrier
    TileContext._drain_patched = True


def _build_nc():
    import concourse.bass as bass
    from concourse import bacc, mybir
    from concourse.tile import TileContext
    from concourse.masks import make_identity
    from contextlib import ExitStack

    _patch_tile_drain()

    f32 = mybir.dt.float32
    f32r = mybir.dt.float32r
    Exp = mybir.ActivationFunctionType.Exp
    mult = mybir.AluOpType.mult
    add = mybir.AluOpType.add

    def r(ap):
        return ap.bitcast(f32r)

    nc = bacc.Bacc()
    xqt = nc.declare_dram_parameter("xqt", [D, LQ], f32, isOutput=False)
    xkt = nc.declare_dram_parameter("xkt", [D, L], f32, isOutput=False)
    xvt = nc.declare_dram_parameter("xvt", [D, L], f32, isOutput=False)
    mb2 = nc.declare_dram_parameter("mb2", [LQ, L], f32, isOutput=False)
    wq = nc.declare_dram_parameter("wq", [D, D], f32, isOutput=False)
    wqr = nc.declare_dram_parameter("wqr", [D, D], f32, isOutput=False)
    wk = nc.declare_dram_parameter("wk", [D, D], f32, isOutput=False)
    wv = nc.declare_dram_parameter("wv", [D, D], f32, isOutput=False)
    wo = nc.declare_dram_parameter("wo", [D, D], f32, isOutput=False)
    bq = nc.declare_dram_parameter("bq", [1, D], f32, isOutput=False)
    bqr = nc.declare_dram_parameter("bqr", [1, D], f32, isOutput=False)
    bk = nc.declare_dram_parameter("bk", [1, D], f32, isOutput=False)
    bv = nc.declare_dram_parameter("bv", [1, D], f32, isOutput=False)
    bo = nc.declare_dram_parameter("bo", [1, D], f32, isOutput=False)
    onesr = nc.declare_dram_parameter("onesr", [1, D], f32, isOutput=False)
    attn_o = nc.declare_dram_parameter("attn", [H, LQ, L], f32, isOutput=True)
    out_o = nc.declare_dram_parameter("out", [LQ, D], f32, isOutput=True)

    with TileContext(nc) as tc, ExitStack() as ctx:
        const = ctx.enter_context(tc.tile_pool(name="const", bufs=1))
        ident = const.tile([128, 128], f32)
        make_identity(nc, ident)
        ones = const.tile([1, D], f32)
        nc.sync.dma_start(r(ones[:]), r(onesr[:]))
        brow = {}
        for nm, t in [("bq", bq), ("bqr", bqr), ("bk", bk), ("bv", bv), ("bo", bo)]:
            rt = const.tile([1, D], f32, tag=f"brow_{nm}")
            nc.sync.dma_start(r(rt[:]), r(t[:]))
            brow[nm] = rt
        # wo_sb[dd, c, o] = Wo[64c+dd, o]
        wo_sb = const.tile([DH, 8, D], f32, tag="wo_sb")
        for c in range(8):
            nc.sync.dma_start(
                r(wo_sb[:, c, :]), r(wo[64 * c : 64 * (c + 1), :])
            )

        for hg in range(2):  # head group: heads 4*hg .. 4*hg+4
            with ExitStack() as gctx:
                pers = gctx.enter_context(
                    tc.tile_pool(name=f"pers{hg}", bufs=1)
                )
                # projT activations, partition = o%128, chunk = o//128
                qpT = pers.tile([128, 4, 512], f32, tag="qpT")
                qpTr = pers.tile([128, 4, 512], f32, tag="qpTr")
                kpT = pers.tile([128, 4, 1024], f32, tag="kpT")
                v_sb = pers.tile([128, 4, 16, DH], f32, tag="v")
                outT = pers.tile([DH, 4, 1024], f32, tag="outT")

                # ---------- projections for this group ----------
                with (
                    tc.tile_pool(name=f"wp{hg}", bufs=1) as wpool,
                    tc.tile_pool(name=f"px{hg}", bufs=3) as projx,
                    tc.tile_pool(name=f"pp{hg}", bufs=2, space="PSUM") as projps,
                    tc.tile_pool(name=f"pv{hg}", bufs=2, space="PSUM") as vtps,
                ):
                    w_sb = {}
                    for nm, wt in [("wq", wq), ("wqr", wqr), ("wk", wk), ("wv", wv)]:
                        ws = wpool.tile([128, 4, D], f32, tag=f"w_{nm}")
                        for j in range(4):
                            nc.sync.dma_start(
                                r(ws[:, j, :]), r(wt[128 * j : 128 * (j + 1), :])
                            )
                        w_sb[nm] = ws
                    vpT = wpool.tile([128, 4, 1024], f32, tag="vpT")

                    def proj(xt, col0, nblk, wnm, bnm, dst):
                        for Bk in range(nblk):
                            xs = projx.tile([128, 4, 256], f32, tag="xs")
                            for j in range(4):
                                nc.sync.dma_start(
                                    r(xs[:, j, :]),
                                    r(
                                        xt[
                                            128 * j : 128 * (j + 1),
                                            col0 + 256 * Bk : col0 + 256 * (Bk + 1),
                                        ]
                                    ),
                                )
                            ps = projps.tile([128, 4, 256], f32, tag="pp")
                            for m in range(4):
                                for j in range(4):
                                    nc.tensor.matmul(
                                        ps[:, m, :],
                                        r(w_sb[wnm][:, j, 128 * m : 128 * (m + 1)]),
                                        r(xs[:, j, :]),
                                        start=(j == 0),
                                        stop=False,
                                    )
                                nc.tensor.matmul(
                                    ps[:, m, :],
                                    r(brow[bnm][0:1, 128 * m : 128 * (m + 1)]),
                                    r(ones[0:1, 0:256]),
                                    start=False,
                                    stop=True,
                                )
                            if Bk % 2 == 0:
                                nc.vector.tensor_copy(
                                    out=r(dst[:, :, 256 * Bk : 256 * (Bk + 1)]),
                                    in_=ps[:],
                                )
                            else:
                                nc.scalar.copy(
                                    out=r(dst[:, :, 256 * Bk : 256 * (Bk + 1)]),
                                    in_=ps[:],
                                )

                    proj(xqt, 512 * hg, 2, "wq", "bq", qpT)
                    proj(xqt, 512 * hg, 2, "wqr", "bqr", qpTr)
                    proj(xkt, 1024 * hg, 4, "wk", "bk", kpT)
                    proj(xvt, 1024 * hg, 4, "wv", "bv", vpT)

                    # v: transpose vpT blocks -> v_sb rows (k' on partitions, blocked)
                    for hl in range(4):
                        for a in range(2):  # j = 8*a + w
                            pv = vtps.tile([128, 512], f32, tag="pv")
                            for w in range(8):
                                j = 8 * a + w
                                ck = CKS[j // 2]
                                pk = ck % 2
                                nc.tensor.transpose(
                                    pv[:, 64 * w : 64 * (w + 1)],
                                    vpT[
                                        64 * pk : 64 * (pk + 1),
                                        ck // 2,
                                        256 * hl + 128 * (j % 2) : 256 * hl
                                        + 128 * (j % 2)
                                        + 128,
                                    ],
                                    ident[64 * pk : 64 * (pk + 1), 64 * pk : 64 * (pk + 1)],
                                )
                            nc.scalar.copy(
                                out=r(v_sb[:, hl, 8 * a : 8 * a + 8, :]),
                                in_=pv.rearrange("p (j d) -> p j d", j=8),
                            )

                # ---------- attention for this group ----------
                with (
                    tc.tile_pool(name=f"mt{hg}", bufs=3) as mtp,
                    tc.tile_pool(name=f"ep{hg}", bufs=3) as epool,
                    tc.tile_pool(name=f"aT{hg}", bufs=2) as aTp,
                    tc.tile_pool(name=f"dn{hg}", bufs=6) as dnp,
                    tc.tile_pool(name=f"os{hg}", bufs=2) as ost,
                    tc.tile_pool(name=f"s{hg}", bufs=2, space="PSUM") as sps,
                    tc.tile_pool(name=f"t{hg}", bufs=2, space="PSUM") as tps,
                    tc.tile_pool(name=f"vt{hg}", bufs=2, space="PSUM") as vtp,
                ):
                    for cqp in range(4):
                        mts = []
                        for u2 in range(2):
                            cq = 2 * cqp + u2
                            mtile = mtp.tile([128, L], f32, tag="mt")
                            nc.sync.dma_start(mtile[:], mb2[cq::8, :])
                            mts.append(mtile)
                        for hl in range(4):
                            h = 4 * hg + hl
                            aT = aTp.tile([128, 16, 256], f32, tag="aT")
                            for u2 in range(2):
                                cq = 2 * cqp + u2
                                e_t = epool.tile([128, L], f32, tag="e")
                                dent = dnp.tile([128, 4], f32, tag="den")
                                for pk in range(2):  # rhs partition half (ck parity)
                                    pss = sps.tile([128, 1024], f32, tag="s")
                                    # lhsT: parity-matched q projT slice
                                    if cq % 2 == pk:
                                        lhs = qpT[
                                            64 * pk : 64 * (pk + 1),
                                            cq // 2,
                                            128 * hl : 128 * (hl + 1),
                                        ]
                                    else:
                                        cq2 = (cq - 1) % 8
                                        lhs = qpTr[
                                            64 * pk : 64 * (pk + 1),
                                            cq2 // 2,
                                            128 * hl : 128 * (hl + 1),
                                        ]
                                    for i2 in range(2):
                                        rhs = kpT[
                                            64 * pk : 64 * (pk + 1),
                                            2 * i2 : 2 * (i2 + 1),
                                            256 * hl : 256 * (hl + 1),
                                        ]
                                        nc.tensor.matmul(
                                            pss[:, 512 * i2 : 512 * (i2 + 1)],
                                            r(lhs),
                                            r(rhs),
                                            start=True,
                                            stop=True,
                                        )
                                    # additive mask (blocked col order matches psum)
                                    nc.vector.tensor_tensor(
                                        out=pss[:],
                                        in0=pss[:],
                                        in1=mts[u2][:, 1024 * pk : 1024 * (pk + 1)],
                                        op=add,
                                    )
                                    # exp: un-permute k order via strided out
                                    eview = e_t.rearrange("p (rk c) -> p c rk", c=8)[
                                        :, pk : 8 : 2, :
                                    ]
                                    nc.scalar.activation(
                                        eview,
                                        pss.rearrange("p (c rk) -> p c rk", rk=256),
                                        Exp,
                                        accum_out=dent[:, pk : pk + 1],
                                    )
                                nc.vector.tensor_tensor(
                                    out=dent[:, 2:3],
                                    in0=dent[:, 0:1],
                                    in1=dent[:, 1:2],
                                    op=add,
                                )
                                nc.vector.reciprocal(dent[:, 3:4], dent[:, 2:3])
                                nc.gpsimd.tensor_scalar(
                                    out=e_t[:],
                                    in0=e_t[:],
                                    scalar1=dent[:, 3:4],
                                    scalar2=None,
                                    op0=mult,
                                )
                                nc.sync.dma_start(attn_o[h, cq::8, :], e_t[:])
                                # transposes of blocked views for the @v matmul
                                ebl = e_t.rearrange("p (rk c) -> p c rk", c=8)
                                for a in range(4):
                                    pt = tps.tile([128, 512], f32, tag="t512")
                                    for w in range(4):
                                        j = 4 * a + w
                                        ck = CKS[j // 2]
                                        nc.tensor.transpose(
                                            pt[:, 128 * w : 128 * (w + 1)],
                                            ebl[:, ck, 128 * (j % 2) : 128 * (j % 2) + 128],
                                            ident[:],
                                        )
                                    cpo = aT[
                                        :, 4 * a : 4 * a + 4, 128 * u2 : 128 * (u2 + 1)
                                    ]
                                    cpi = pt.rearrange("p (j q) -> p j q", j=4)
                                    if a % 2 == 0:
                                        nc.scalar.copy(out=r(cpo), in_=cpi)
                                    else:
                                        nc.vector.tensor_copy(out=r(cpo), in_=cpi)
                            # attn @ v for (head, cq pair)
                            pvt = vtp.tile([DH, 256], f32, tag="pvt")
                            for j in range(16):
                                nc.tensor.matmul(
                                    pvt[:],
                                    r(v_sb[:, hl, j, :]),
                                    r(aT[:, j, :]),
                                    start=(j == 0),
                                    stop=(j == 15),
                                )
                            # outT[d', hl, 8*rq + 2*cqp + u2] <- pvt[d', 128u2+rq]
                            dstT = outT[:, hl, :].rearrange(
                                "p (rq c) -> p c rq", c=8
                            )[:, 2 * cqp : 2 * cqp + 2, :]
                            nc.scalar.copy(
                                out=r(dstT),
                                in_=pvt.rearrange("p (c rq) -> p c rq", rq=128),
                            )

                    # output projection for this group
                    for hl in range(4):
                        h = 4 * hg + hl
                        po = tps.tile([128, 512], f32, tag="t512")
                        for c in range(8):
                            nc.tensor.matmul(
                                po[:],
                                r(outT[:, hl, c::8]),
                                r(wo_sb[:, c, :]),
                                start=(c == 0),
                                stop=False,
                            )
                        nc.tensor.matmul(
                            po[:],
                            r(ones[0:1, 0:128]),
                            r(brow["bo"][0:1, :]),
                            start=False,
                            stop=True,
                        )
                        o_sb = ost.tile([128, D], f32, tag="ost")
                        nc.scalar.copy(out=o_sb[:], in_=po[:])
                        nc.sync.dma_start(out_o[128 * h : 128 * (h + 1), :], o_sb[:])

    nc.finalize()
    return nc


def _get_nc():
    if "nc" not in _CACHE:
        _CACHE["nc"] = _build_nc()
    return _CACHE["nc"]


def _host_prep(inputs):
    Q = np.asarray(inputs["Q"], dtype=np.float32)
    K = np.asarray(inputs["K"], dtype=np.float32)
    V = np.asarray(inputs["V"], dtype=np.float32)
    mask = np.asarray(inputs["mask"])
    Wq = np.asarray(inputs["Wq"], dtype=np.float32) / 32.0
    bqv = np.asarray(inputs["bq"], dtype=np.float32) / 32.0
    Wqr = np.roll(Wq, -64, axis=1)
    bqr = np.roll(bqv, -64)
    com = {
        "wq": np.ascontiguousarray(Wq),
        "wqr": np.ascontiguousarray(Wqr),
        "wk": np.ascontiguousarray(np.asarray(inputs["Wk"], dtype=np.float32)),
        "wv": np.ascontiguousarray(np.asarray(inputs["Wv"], dtype=np.float32)),
        "wo": np.ascontiguousarray(np.asarray(inputs["Wo"], dtype=np.float32)),
        "bq": np.ascontiguousarray(bqv.reshape(1, D)),
        "bqr": np.ascontiguousarray(bqr.reshape(1, D)),
        "bk": np.ascontiguousarray(np.asarray(inputs["bk"], dtype=np.float32).reshape(1, D)),
        "bv": np.ascontiguousarray(np.asarray(inputs["bv"], dtype=np.float32).reshape(1, D)),
        "bo": np.ascontiguousarray(np.asarray(inputs["bo"], dtype=np.float32).reshape(1, D)),
        "onesr": np.ones((1, D), np.float32),
    }
    qrows = np.concatenate([np.arange(256 * h, 256 * h + 128) for h in range(H)])
    in_maps = []
    for c in range(NCORES):
        b, g = c // 2, c % 2
        mslice = mask[b, 0, 1024 * g : 1024 * (g + 1), :].astype(np.float32)
        mbias = (mslice - 1.0) * 1e9
        # permute k columns to the blocked order the scores psum produces
        mb2v = mbias.reshape(LQ, 256, 8)[:, :, CKS].transpose(0, 2, 1).reshape(LQ, L)
        in_maps.append(
            {
                "xqt": np.ascontiguousarray(Q[b][qrows + 128 * g].T),
                "xkt": np.ascontiguousarray(K[b].T),
                "xvt": np.ascontiguousarray(V[b].T),
                "mb2": np.ascontiguousarray(mb2v),
                **com,
            }
        )
    return in_maps


def kernel(**inputs):
    from concourse.bass_utils import run_bass_kernel_spmd

    nc = _get_nc()
    in_maps = _host_prep(inputs)
    res = run_bass_kernel_spmd(nc, in_maps, list(range(NCORES))).results

    out = np.empty((B, L, D), np.float32)
    attn = np.empty((B, H, L, L), np.float32)
    for c in range(NCORES):
        b, g = c // 2, c % 2
        attn[b, :, 1024 * g : 1024 * (g + 1), :] = res[c]["attn"]
        o = res[c]["out"]
        for h in range(H):
            out[b, 256 * h + 128 * g : 256 * h + 128 * g + 128, :] = o[
                128 * h : 128 * (h + 1)
            ]
    return out, attn


# revision 9
# speedup vs baseline: 3.1758x; 3.1758x over previous
"""Bass/Trainium2 kernel for nn_Attention_481036337480.

Sharding: 8 cores = 4 batches x 2 query-halves. Core c handles batch
b=c//2 and query rows l' in [1024g, 1024(g+1)) (g=c%2) for ALL 8 heads.
The reference's raw reshape (B,L,D)->(B,H,L,dh) means head h only
involves projection rows [256h, 256h+256), so every per-core output
slice (attn_dist rows, out rows) is disjoint -> no collectives.

Layout trick: the reshape interleave l' = 8*r + c means head-h q/k/v
transposed slices are NOT plain APs of proj^T. Instead of de-interleaving
(which would need cross-partition copies the engines can't do), scores
are computed directly from projT psum layout as (cq, ck) 64-contraction
sub-blocks; a host-rolled copy of Wq supplies lhsT at the opposite
partition base for parity-mismatched (cq, ck) pairs. The blocked k-order
is un-permuted for free inside the strided-output exp pass.

Softmax: additive mask bias (host: (m-1)*1e9, k-block-permuted cols),
exp on ScalarE with accum_out denominators, normalize on GPSIMD.
"""

import sys

sys.path.insert(0, "/opt/trn_rl_repo")

import numpy as np

B, L, D, H = 4, 2048, 512, 8
DH = D // H  # 64
LQ = 1024  # q rows per core
NCORES = 8
CKS = [0, 2, 4, 6, 1, 3, 5, 7]  # k-block order as produced in scores psum

_CACHE = {}


def _patch_tile_drain():
    """walrus CTRL instructions accept only 1 sem wait; Tile's final drain
    attaches one wait per still-open semaphore. Chunk across chained drains."""
    from concourse.tile import TileContext
    from concourse.vector_clock import ScopedClock
    from concourse import mybir

    if getattr(TileContext, "_drain_patched", False):
        return

    def _drain_and_barrier(self, tick_clock, wait_clock):
        nc = self.nc
        drain_inst = nc.sync.drain()
        wait_clock.add_sem_waits(
            drain_inst.ins, ScopedClock({None: tick_clock.global_clock})
        )
        si = drain_inst.ins.sync_info
        waits = list(si.on_wait or [])
        if len(waits) > 1:
            si.on_wait.clear()
            si.on_wait.append(waits[0])
            for w in waits[1:]:
                extra = nc.sync.drain()
                if extra.ins.sync_info is None:
                    extra.ins.sync_info = mybir.SyncInfo(on_wait=[], on_update=[])
                extra.ins.sync_info.on_wait.append(w)
        nc.all_engine_barrier()
        assert self.sems is not None
        popped = nc._tile_sem_poison_stack.pop()
        assert popped is self._sem_poison
        nc.clear_and_free_semaphores(list(self.sems.allocated().values()))
        nc.all_engine_barrier()

    TileContext._drain_and_barrier = _drain_and_barrier
    TileContext._drain_patched = True


def _build_nc():
    import concourse.bass as bass
    from concourse import bacc, mybir
    from concourse.tile import TileContext
    from concourse.masks import make_identity
    from contextlib import ExitStack

    _patch_tile_drain()

    f32 = mybir.dt.float32
    f32r = mybir.dt.float32r
    Exp = mybir.ActivationFunctionType.Exp
    mult = mybir.AluOpType.mult
    add = mybir.AluOpType.add

    def r(ap):
        return ap.bitcast(f32r)

    nc = bacc.Bacc()
    xqt = nc.declare_dram_parameter("xqt", [D, LQ], f32, isOutput=False)
    xkt = nc.declare_dram_parameter("xkt", [D, L], f32, isOutput=False)
    xvt = nc.declare_dram_parameter("xvt", [D, L], f32, isOutput=False)
    mb2 = nc.declare_dram_parameter("mb2", [LQ, L], f32, isOutput=False)
    wq = nc.declare_dram_parameter("wq", [D, D], f32, isOutput=False)
    wqr = nc.declare_dram_parameter("wqr", [D, D], f32, isOutput=False)
    wk = nc.declare_dram_parameter("wk", [D, D], f32, isOutput=False)
    wv = nc.declare_dram_parameter("wv", [D, D], f32, isOutput=False)
    wo = nc.declare_dram_parameter("wo", [D, D], f32, isOutput=False)
    bq = nc.declare_dram_parameter("bq", [1, D], f32, isOutput=False)
    bqr = nc.declare_dram_parameter("bqr", [1, D], f32, isOutput=False)
    bk = nc.declare_dram_parameter("bk", [1, D], f32, isOutput=False)
    bv = nc.declare_dram_parameter("bv", [1, D], f32, isOutput=False)
    bo = nc.declare_dram_parameter("bo", [1, D], f32, isOutput=False)
    onesr = nc.declare_dram_parameter("onesr", [1, D], f32, isOutput=False)
    identr = nc.declare_dram_parameter("identr", [128, 128], f32, isOutput=False)
    attn_o = nc.declare_dram_parameter("attn", [H, LQ, L], f32, isOutput=True)
    out_o = nc.declare_dram_parameter("out", [LQ, D], f32, isOutput=True)

    with TileContext(nc) as tc, ExitStack() as ctx:
        const = ctx.enter_context(tc.tile_pool(name="const", bufs=1))
        ident = const.tile([128, 128], f32)
        nc.sync.dma_start(r(ident[:]), r(identr[:]))
        ones = const.tile([1, D], f32)
        nc.sync.dma_start(r(ones[:]), r(onesr[:]))
        brow = {}
        for nm, t in [("bq", bq), ("bqr", bqr), ("bk", bk), ("bv", bv), ("bo", bo)]:
            rt = const.tile([1, D], f32, tag=f"brow_{nm}")
            nc.sync.dma_start(r(rt[:]), r(t[:]))
            brow[nm] = rt
        # wo_sb[dd, c, o] = Wo[64c+dd, o]
        wo_sb = const.tile([DH, 8, D], f32, tag="wo_sb")
        for c in range(8):
            nc.sync.dma_start(
                r(wo_sb[:, c, :]), r(wo[64 * c : 64 * (c + 1), :])
            )

        for hg in range(2):  # head group: heads 4*hg .. 4*hg+4
            with ExitStack() as gctx:
                pers = gctx.enter_context(
                    tc.tile_pool(name=f"pers{hg}", bufs=1)
                )
                # projT activations, partition = o%128, chunk = o//128
                qpT = pers.tile([128, 4, 512], f32, tag="qpT")
                qpTr = pers.tile([128, 4, 512], f32, tag="qpTr")
                kpT = pers.tile([128, 4, 1024], f32, tag="kpT")
                v_sb = pers.tile([128, 4, 16, DH], f32, tag="v")
                outT = pers.tile([DH, 4, 1024], f32, tag="outT")

                # ---------- projections for this group ----------
                with (
                    tc.tile_pool(name=f"wp{hg}", bufs=1) as wpool,
                    tc.tile_pool(name=f"px{hg}", bufs=3) as projx,
                    tc.tile_pool(name=f"pp{hg}", bufs=2, space="PSUM") as projps,
                    tc.tile_pool(name=f"pv{hg}", bufs=2, space="PSUM") as vtps,
                ):
                    w_sb = {}
                    for nm, wt in [("wq", wq), ("wqr", wqr), ("wk", wk), ("wv", wv)]:
                        ws = wpool.tile([128, 4, D], f32, tag=f"w_{nm}")
                        for j in range(4):
                            nc.sync.dma_start(
                                r(ws[:, j, :]), r(wt[128 * j : 128 * (j + 1), :])
                            )
                        w_sb[nm] = ws
                    vpT = wpool.tile([128, 4, 1024], f32, tag="vpT")

                    def proj(xt, col0, nblk, wnm, bnm, dst):
                        for Bk in range(nblk):
                            xs = projx.tile([128, 4, 256], f32, tag="xs")
                            for j in range(4):
                                nc.sync.dma_start(
                                    r(xs[:, j, :]),
                                    r(
                                        xt[
                                            128 * j : 128 * (j + 1),
                                            col0 + 256 * Bk : col0 + 256 * (Bk + 1),
                                        ]
                                    ),
                                )
                            ps = projps.tile([128, 4, 256], f32, tag="pp")
                            for m in range(4):
                                for j in range(4):
                                    nc.tensor.matmul(
                                        ps[:, m, :],
                                        r(w_sb[wnm][:, j, 128 * m : 128 * (m + 1)]),
                                        r(xs[:, j, :]),
                                        start=(j == 0),
                                        stop=False,
                                    )
                                nc.tensor.matmul(
                                    ps[:, m, :],
                                    r(brow[bnm][0:1, 128 * m : 128 * (m + 1)]),
                                    r(ones[0:1, 0:256]),
                                    start=False,
                                    stop=True,
                                )
                            if Bk % 2 == 0:
                                nc.vector.tensor_copy(
                                    out=r(dst[:, :, 256 * Bk : 256 * (Bk + 1)]),
                                    in_=ps[:],
                                )
                            else:
                                nc.scalar.copy(
                                    out=r(dst[:, :, 256 * Bk : 256 * (Bk + 1)]),
                                    in_=ps[:],
                                )

                    proj(xqt, 512 * hg, 2, "wq", "bq", qpT)
                    proj(xqt, 512 * hg, 2, "wqr", "bqr", qpTr)
                    proj(xkt, 1024 * hg, 4, "wk", "bk", kpT)
                    proj(xvt, 1024 * hg, 4, "wv", "bv", vpT)

                    # v: transpose vpT blocks -> v_sb rows (k' on partitions, blocked)
                    for hl in range(4):
                        for a in range(2):  # j = 8*a + w
                            pv = vtps.tile([128, 512], f32, tag="pv")
                            for w in range(8):
                                j = 8 * a + w
                                ck = CKS[j // 2]
                                pk = ck % 2
                                nc.tensor.transpose(
                                    r(pv[:, 64 * w : 64 * (w + 1)]),
                                    r(
                                        vpT[
                                            64 * pk : 64 * (pk + 1),
                                            ck // 2,
                                            256 * hl + 128 * (j % 2) : 256 * hl
                                            + 128 * (j % 2)
                                            + 128,
                                        ]
                                    ),
                                    r(
                                        ident[
                                            64 * pk : 64 * (pk + 1),
                                            64 * pk : 64 * (pk + 1),
                                        ]
                                    ),
                                )
                            nc.scalar.copy(
                                out=r(v_sb[:, hl, 8 * a : 8 * a + 8, :]),
                                in_=pv.rearrange("p (j d) -> p j d", j=8),
                            )

                # ---------- attention for this group ----------
                with (
                    tc.tile_pool(name=f"mt{hg}", bufs=2) as mtp,
                    tc.tile_pool(name=f"sm{hg}", bufs=2) as smp,
                    tc.tile_pool(name=f"ep{hg}", bufs=3) as epool,
                    tc.tile_pool(name=f"aT{hg}", bufs=2) as aTp,
                    tc.tile_pool(name=f"dn{hg}", bufs=6) as dnp,
                    tc.tile_pool(name=f"os{hg}", bufs=2) as ost,
                    tc.tile_pool(name=f"s{hg}", bufs=2, space="PSUM") as sps,
                    tc.tile_pool(name=f"t{hg}", bufs=2, space="PSUM") as tps,
                    tc.tile_pool(name=f"vt{hg}", bufs=2, space="PSUM") as vtp,
                ):
                    for cqp in range(4):
                        mts = []
                        for u2 in range(2):
                            cq = 2 * cqp + u2
                            mtile = mtp.tile([128, L], f32, tag="mt")
                            nc.sync.dma_start(mtile[:], mb2[cq::8, :])
                            mts.append(mtile)
                        for hl in range(4):
                            h = 4 * hg + hl
                            aT = aTp.tile([128, 16, 256], f32, tag="aT")
                            for u2 in range(2):
                                cq = 2 * cqp + u2
                                e_t = epool.tile([128, L], f32, tag="e")
                                sm_t = smp.tile([128, L], f32, tag="sm")
                                dent = dnp.tile([128, 4], f32, tag="den")
                                for pk in range(2):  # rhs partition half (ck parity)
                                    pss = sps.tile([128, 1024], f32, tag="s")
                                    # lhsT: parity-matched q projT slice
                                    if cq % 2 == pk:
                                        lhs = qpT[
                                            64 * pk : 64 * (pk + 1),
                                            cq // 2,
                                            128 * hl : 128 * (hl + 1),
                                        ]
                                    else:
                                        cq2 = (cq - 1) % 8
                                        lhs = qpTr[
                                            64 * pk : 64 * (pk + 1),
                                            cq2 // 2,
                                            128 * hl : 128 * (hl + 1),
                                        ]
                                    for i2 in range(2):
                                        rhs = kpT[
                                            64 * pk : 64 * (pk + 1),
                                            2 * i2 : 2 * (i2 + 1),
                                            256 * hl : 256 * (hl + 1),
                                        ]
                                        nc.tensor.matmul(
                                            pss[:, 512 * i2 : 512 * (i2 + 1)],
                                            r(lhs),
                                            r(rhs),
                                            start=True,
                                            stop=True,
                                        )
                                    # additive mask (blocked col order matches psum)
                                    nc.vector.tensor_tensor(
                                        out=sm_t[:, 1024 * pk : 1024 * (pk + 1)],
                                        in0=pss[:],
                                        in1=mts[u2][:, 1024 * pk : 1024 * (pk + 1)],
                                        op=add,
                                    )
                                    # exp: un-permute k order via strided out
                                    eview = e_t.rearrange("p (rk c) -> p c rk", c=8)[
                                        :, pk : 8 : 2, :
                                    ]
                                    nc.scalar.activation(
                                        r(eview),
                                        sm_t[
                                            :, 1024 * pk : 1024 * (pk + 1)
                                        ].rearrange("p (c rk) -> p c rk", rk=256),
                                        Exp,
                                        accum_out=dent[:, pk : pk + 1],
                                    )
                                nc.vector.tensor_tensor(
                                    out=dent[:, 2:3],
                                    in0=dent[:, 0:1],
                                    in1=dent[:, 1:2],
                                    op=add,
                                )
                                nc.vector.reciprocal(dent[:, 3:4], dent[:, 2:3])
                                nc.vector.tensor_scalar(
                                    out=r(e_t[:]),
                                    in0=e_t[:],
                                    scalar1=dent[:, 3:4],
                                    scalar2=None,
                                    op0=mult,
                                )
                                nc.sync.dma_start(attn_o[h, cq::8, :], e_t[:])
                                # transposes of blocked views for the @v matmul
                                ebl = e_t.rearrange("p (rk c) -> p c rk", c=8)
                                for a in range(4):
                                    pt = tps.tile([128, 512], f32, tag="t512")
                                    for w in range(4):
                                        j = 4 * a + w
                                        ck = CKS[j // 2]
                                        nc.tensor.transpose(
                                            r(pt[:, 128 * w : 128 * (w + 1)]),
                                            r(
                                                ebl[
                                                    :,
                                                    ck,
                                                    128 * (j % 2) : 128 * (j % 2) + 128,
                                                ]
                                            ),
                                            r(ident[:]),
                                        )
                                    cpo = aT[
                                        :, 4 * a : 4 * a + 4, 128 * u2 : 128 * (u2 + 1)
                                    ]
                                    cpi = pt.rearrange("p (j q) -> p j q", j=4)
                                    if a % 2 == 0:
                                        nc.scalar.copy(out=r(cpo), in_=cpi)
                                    else:
                                        nc.vector.tensor_copy(out=r(cpo), in_=cpi)
                            # attn @ v for (head, cq pair)
                            pvt = vtp.tile([DH, 256], f32, tag="pvt")
                            for j in range(16):
                                nc.tensor.matmul(
                                    pvt[:],
                                    r(v_sb[:, hl, j, :]),
                                    r(aT[:, j, :]),
                                    start=(j == 0),
                                    stop=(j == 15),
                                )
                            # outT[d', hl, 8*rq + 2*cqp + u2] <- pvt[d', 128u2+rq]
                            dstT = outT[:, hl, :].rearrange(
                                "p (rq c) -> p c rq", c=8
                            )[:, 2 * cqp : 2 * cqp + 2, :]
                            nc.scalar.copy(
                                out=r(dstT),
                                in_=pvt.rearrange("p (c rq) -> p c rq", rq=128),
                            )

                    # output projection for this group
                    for hl in range(4):
                        h = 4 * hg + hl
                        po = tps.tile([128, 512], f32, tag="t512")
                        for c in range(8):
                            nc.tensor.matmul(
                                po[:],
                                r(outT[:, hl, c::8]),
                                r(wo_sb[:, c, :]),
                                start=(c == 0),
                                stop=False,
                            )
                        nc.tensor.matmul(
                            po[:],
                            r(ones[0:1, 0:128]),
                            r(brow["bo"][0:1, :]),
                            start=False,
                            stop=True,
                        )
                        o_sb = ost.tile([128, D], f32, tag="ost")
                        nc.scalar.copy(out=o_sb[:], in_=po[:])
                        nc.sync.dma_start(out_o[128 * h : 128 * (h + 1), :], o_sb[:])

    nc.finalize()
    return nc


def _get_nc():
    if "nc" not in _CACHE:
        _CACHE["nc"] = _build_nc()
    return _CACHE["nc"]


def _host_prep(inputs):
    Q = np.asarray(inputs["Q"], dtype=np.float32)
    K = np.asarray(inputs["K"], dtype=np.float32)
    V = np.asarray(inputs["V"], dtype=np.float32)
    mask = np.asarray(inputs["mask"])
    Wq = np.asarray(inputs["Wq"], dtype=np.float32) / 32.0
    bqv = np.asarray(inputs["bq"], dtype=np.float32) / 32.0
    Wqr = np.roll(Wq, -64, axis=1)
    bqr = np.roll(bqv, -64)
    com = {
        "wq": np.ascontiguousarray(Wq),
        "wqr": np.ascontiguousarray(Wqr),
        "wk": np.ascontiguousarray(np.asarray(inputs["Wk"], dtype=np.float32)),
        "wv": np.ascontiguousarray(np.asarray(inputs["Wv"], dtype=np.float32)),
        "wo": np.ascontiguousarray(np.asarray(inputs["Wo"], dtype=np.float32)),
        "bq": np.ascontiguousarray(bqv.reshape(1, D)),
        "bqr": np.ascontiguousarray(bqr.reshape(1, D)),
        "bk": np.ascontiguousarray(np.asarray(inputs["bk"], dtype=np.float32).reshape(1, D)),
        "bv": np.ascontiguousarray(np.asarray(inputs["bv"], dtype=np.float32).reshape(1, D)),
        "bo": np.ascontiguousarray(np.asarray(inputs["bo"], dtype=np.float32).reshape(1, D)),
        "onesr": np.ones((1, D), np.float32),
        "identr": np.eye(128, dtype=np.float32),
    }
    qrows = np.concatenate([np.arange(256 * h, 256 * h + 128) for h in range(H)])
    in_maps = []
    for c in range(NCORES):
        b, g = c // 2, c % 2
        mslice = mask[b, 0, 1024 * g : 1024 * (g + 1), :].astype(np.float32)
        mbias = (mslice - 1.0) * 1e9
        # permute k columns to the blocked order the scores psum produces
        mb2v = mbias.reshape(LQ, 256, 8)[:, :, CKS].transpose(0, 2, 1).reshape(LQ, L)
        in_maps.append(
            {
                "xqt": np.ascontiguousarray(Q[b][qrows + 128 * g].T),
                "xkt": np.ascontiguousarray(K[b].T),
                "xvt": np.ascontiguousarray(V[b].T),
                "mb2": np.ascontiguousarray(mb2v),
                **com,
            }
        )
    return in_maps


def kernel(**inputs):
    from concourse.bass_utils import run_bass_kernel_spmd

    nc = _get_nc()
    in_maps = _host_prep(inputs)
    res = run_bass_kernel_spmd(nc, in_maps, list(range(NCORES))).results

    out = np.empty((B, L, D), np.float32)
    attn = np.empty((B, H, L, L), np.float32)
    for c in range(NCORES):
        b, g = c // 2, c % 2
        attn[b, :, 1024 * g : 1024 * (g + 1), :] = res[c]["attn"]
        o = res[c]["out"]
        for h in range(H):
            out[b, 256 * h + 128 * g : 256 * h + 128 * g + 128, :] = o[
                128 * h : 128 * (h + 1)
            ]
    return out, attn


# revision 10
# speedup vs baseline: 3.2260x; 1.0158x over previous
"""Bass/Trainium2 kernel for nn_Attention_481036337480.

Sharding: 8 cores = 4 batches x 2 query-halves. Core c handles batch
b=c//2 and query rows l' in [1024g, 1024(g+1)) (g=c%2) for ALL 8 heads.
The reference's raw reshape (B,L,D)->(B,H,L,dh) means head h only
involves projection rows [256h, 256h+256), so every per-core output
slice (attn_dist rows, out rows) is disjoint -> no collectives.

Layout trick: the reshape interleave l' = 8*r + c means head-h q/k/v
transposed slices are NOT plain APs of proj^T. Instead of de-interleaving
(which would need cross-partition copies the engines can't do), scores
are computed directly from projT psum layout as (cq, ck) 64-contraction
sub-blocks; a host-rolled copy of Wq supplies lhsT at the opposite
partition base for parity-mismatched (cq, ck) pairs. The blocked k-order
is un-permuted for free inside the strided-output exp pass.

Softmax: additive mask bias (host: (m-1)*1e9, k-block-permuted cols),
exp on ScalarE with accum_out denominators, normalize on GPSIMD.
"""

import sys

sys.path.insert(0, "/opt/trn_rl_repo")

import numpy as np

B, L, D, H = 4, 2048, 512, 8
DH = D // H  # 64
LQ = 1024  # q rows per core
NCORES = 8
CKS = [0, 2, 4, 6, 1, 3, 5, 7]  # k-block order as produced in scores psum

_CACHE = {}


def _patch_tile_drain():
    """walrus CTRL instructions accept only 1 sem wait; Tile's final drain
    attaches one wait per still-open semaphore. Chunk across chained drains."""
    from concourse.tile import TileContext
    from concourse.vector_clock import ScopedClock
    from concourse import mybir

    if getattr(TileContext, "_drain_patched", False):
        return

    def _drain_and_barrier(self, tick_clock, wait_clock):
        nc = self.nc
        drain_inst = nc.sync.drain()
        wait_clock.add_sem_waits(
            drain_inst.ins, ScopedClock({None: tick_clock.global_clock})
        )
        si = drain_inst.ins.sync_info
        waits = list(si.on_wait or [])
        if len(waits) > 1:
            si.on_wait.clear()
            si.on_wait.append(waits[0])
            for w in waits[1:]:
                extra = nc.sync.drain()
                if extra.ins.sync_info is None:
                    extra.ins.sync_info = mybir.SyncInfo(on_wait=[], on_update=[])
                extra.ins.sync_info.on_wait.append(w)
        nc.all_engine_barrier()
        assert self.sems is not None
        popped = nc._tile_sem_poison_stack.pop()
        assert popped is self._sem_poison
        nc.clear_and_free_semaphores(list(self.sems.allocated().values()))
        nc.all_engine_barrier()

    TileContext._drain_and_barrier = _drain_and_barrier
    TileContext._drain_patched = True


def _build_nc():
    import concourse.bass as bass
    from concourse import bacc, mybir
    from concourse.tile import TileContext
    from concourse.masks import make_identity
    from contextlib import ExitStack

    _patch_tile_drain()

    f32 = mybir.dt.float32
    f32r = mybir.dt.float32r
    bf16 = mybir.dt.bfloat16
    Exp = mybir.ActivationFunctionType.Exp
    mult = mybir.AluOpType.mult
    add = mybir.AluOpType.add

    def r(ap):
        return ap.bitcast(f32r)

    nc = bacc.Bacc()
    xqt = nc.declare_dram_parameter("xqt", [D, LQ], f32, isOutput=False)
    xkt = nc.declare_dram_parameter("xkt", [D, L], f32, isOutput=False)
    xvt = nc.declare_dram_parameter("xvt", [D, L], f32, isOutput=False)
    mb2 = nc.declare_dram_parameter("mb2", [LQ, L], f32, isOutput=False)
    wq = nc.declare_dram_parameter("wq", [D, D], f32, isOutput=False)
    wqr = nc.declare_dram_parameter("wqr", [D, D], f32, isOutput=False)
    wk = nc.declare_dram_parameter("wk", [D, D], f32, isOutput=False)
    wv = nc.declare_dram_parameter("wv", [D, D], f32, isOutput=False)
    wo = nc.declare_dram_parameter("wo", [D, D], f32, isOutput=False)
    bq = nc.declare_dram_parameter("bq", [1, D], f32, isOutput=False)
    bqr = nc.declare_dram_parameter("bqr", [1, D], f32, isOutput=False)
    bk = nc.declare_dram_parameter("bk", [1, D], f32, isOutput=False)
    bv = nc.declare_dram_parameter("bv", [1, D], f32, isOutput=False)
    bo = nc.declare_dram_parameter("bo", [1, D], f32, isOutput=False)
    onesr = nc.declare_dram_parameter("onesr", [1, D], f32, isOutput=False)
    identr = nc.declare_dram_parameter("identr", [128, 128], f32, isOutput=False)
    identb = nc.declare_dram_parameter("identb", [128, 128], mybir.dt.bfloat16, isOutput=False)
    attn_o = nc.declare_dram_parameter("attn", [H, LQ, L], f32, isOutput=True)
    out_o = nc.declare_dram_parameter("out", [LQ, D], f32, isOutput=True)

    with TileContext(nc) as tc, ExitStack() as ctx:
        const = ctx.enter_context(tc.tile_pool(name="const", bufs=1))
        ident = const.tile([128, 128], f32)
        nc.sync.dma_start(r(ident[:]), r(identr[:]))
        identb_sb = const.tile([128, 128], bf16, tag="identb")
        nc.sync.dma_start(identb_sb[:], identb[:])
        ones = const.tile([1, D], f32)
        nc.sync.dma_start(r(ones[:]), r(onesr[:]))
        brow = {}
        for nm, t in [("bq", bq), ("bqr", bqr), ("bk", bk), ("bv", bv), ("bo", bo)]:
            rt = const.tile([1, D], f32, tag=f"brow_{nm}")
            nc.sync.dma_start(r(rt[:]), r(t[:]))
            brow[nm] = rt
        # wo_sb[dd, c, o] = Wo[64c+dd, o]
        wo_sb = const.tile([DH, 8, D], f32, tag="wo_sb")
        for c in range(8):
            nc.sync.dma_start(
                r(wo_sb[:, c, :]), r(wo[64 * c : 64 * (c + 1), :])
            )

        for hg in range(2):  # head group: heads 4*hg .. 4*hg+4
            with ExitStack() as gctx:
                pers = gctx.enter_context(
                    tc.tile_pool(name=f"pers{hg}", bufs=1)
                )
                # projT activations, partition = o%128, chunk = o//128
                qpT = pers.tile([128, 4, 512], bf16, tag="qpT")
                qpTr = pers.tile([128, 4, 512], bf16, tag="qpTr")
                kpT = pers.tile([128, 4, 1024], bf16, tag="kpT")
                v_sb = pers.tile([128, 4, 16, DH], bf16, tag="v")
                outT = pers.tile([DH, 4, 1024], f32, tag="outT")

                # ---------- projections for this group ----------
                with (
                    tc.tile_pool(name=f"wp{hg}", bufs=1) as wpool,
                    tc.tile_pool(name=f"px{hg}", bufs=3) as projx,
                    tc.tile_pool(name=f"pp{hg}", bufs=2, space="PSUM") as projps,
                    tc.tile_pool(name=f"pv{hg}", bufs=2, space="PSUM") as vtps,
                ):
                    w_sb = {}
                    for nm, wt in [("wq", wq), ("wqr", wqr), ("wk", wk), ("wv", wv)]:
                        ws = wpool.tile([128, 4, D], f32, tag=f"w_{nm}")
                        for j in range(4):
                            nc.sync.dma_start(
                                r(ws[:, j, :]), r(wt[128 * j : 128 * (j + 1), :])
                            )
                        w_sb[nm] = ws
                    vpT = wpool.tile([128, 4, 1024], f32, tag="vpT")

                    def proj(xt, col0, nblk, wnm, bnm, dst):
                        for Bk in range(nblk):
                            xs = projx.tile([128, 4, 256], f32, tag="xs")
                            for j in range(4):
                                nc.sync.dma_start(
                                    r(xs[:, j, :]),
                                    r(
                                        xt[
                                            128 * j : 128 * (j + 1),
                                            col0 + 256 * Bk : col0 + 256 * (Bk + 1),
                                        ]
                                    ),
                                )
                            ps = projps.tile([128, 4, 256], f32, tag="pp")
                            for m in range(4):
                                for j in range(4):
                                    nc.tensor.matmul(
                                        ps[:, m, :],
                                        r(w_sb[wnm][:, j, 128 * m : 128 * (m + 1)]),
                                        r(xs[:, j, :]),
                                        start=(j == 0),
                                        stop=False,
                                    )
                                nc.tensor.matmul(
                                    ps[:, m, :],
                                    r(brow[bnm][0:1, 128 * m : 128 * (m + 1)]),
                                    r(ones[0:1, 0:256]),
                                    start=False,
                                    stop=True,
                                )
                            dv = dst[:, :, 256 * Bk : 256 * (Bk + 1)]
                            if dst.dtype != bf16:
                                dv = r(dv)
                            if Bk % 2 == 0:
                                nc.vector.tensor_copy(out=dv, in_=ps[:])
                            else:
                                nc.scalar.copy(out=dv, in_=ps[:])

                    proj(xqt, 512 * hg, 2, "wq", "bq", qpT)
                    proj(xqt, 512 * hg, 2, "wqr", "bqr", qpTr)
                    proj(xkt, 1024 * hg, 4, "wk", "bk", kpT)
                    proj(xvt, 1024 * hg, 4, "wv", "bv", vpT)

                    # v: transpose vpT blocks -> v_sb rows (k' on partitions, blocked)
                    for hl in range(4):
                        for a in range(2):  # j = 8*a + w
                            pv = vtps.tile([128, 512], f32, tag="pv")
                            for w in range(8):
                                j = 8 * a + w
                                ck = CKS[j // 2]
                                pk = ck % 2
                                nc.tensor.transpose(
                                    r(pv[:, 64 * w : 64 * (w + 1)]),
                                    r(
                                        vpT[
                                            64 * pk : 64 * (pk + 1),
                                            ck // 2,
                                            256 * hl + 128 * (j % 2) : 256 * hl
                                            + 128 * (j % 2)
                                            + 128,
                                        ]
                                    ),
                                    r(
                                        ident[
                                            64 * pk : 64 * (pk + 1),
                                            64 * pk : 64 * (pk + 1),
                                        ]
                                    ),
                                )
                            nc.scalar.copy(
                                out=v_sb[:, hl, 8 * a : 8 * a + 8, :],
                                in_=pv.rearrange("p (j d) -> p j d", j=8),
                            )

                # ---------- attention for this group ----------
                with (
                    tc.tile_pool(name=f"mt{hg}", bufs=2) as mtp,
                    tc.tile_pool(name=f"sm{hg}", bufs=2) as smp,
                    tc.tile_pool(name=f"ep{hg}", bufs=3) as epool,
                    tc.tile_pool(name=f"ab{hg}", bufs=2) as abp,
                    tc.tile_pool(name=f"aT{hg}", bufs=2) as aTp,
                    tc.tile_pool(name=f"dn{hg}", bufs=6) as dnp,
                    tc.tile_pool(name=f"os{hg}", bufs=2) as ost,
                    tc.tile_pool(name=f"s{hg}", bufs=2, space="PSUM") as sps,
                    tc.tile_pool(name=f"t{hg}", bufs=2, space="PSUM") as tps,
                    tc.tile_pool(name=f"vt{hg}", bufs=2, space="PSUM") as vtp,
                ):
                    for cqp in range(4):
                        mts = []
                        for u2 in range(2):
                            cq = 2 * cqp + u2
                            mtile = mtp.tile([128, L], f32, tag="mt")
                            nc.sync.dma_start(mtile[:], mb2[cq::8, :])
                            mts.append(mtile)
                        for hl in range(4):
                            h = 4 * hg + hl
                            aT = aTp.tile([128, 16, 256], bf16, tag="aT")
                            for u2 in range(2):
                                cq = 2 * cqp + u2
                                e_t = epool.tile([128, L], f32, tag="e")
                                sm_t = smp.tile([128, L], f32, tag="sm")
                                dent = dnp.tile([128, 4], f32, tag="den")
                                for pk in range(2):  # rhs partition half (ck parity)
                                    pss = sps.tile([128, 1024], f32, tag="s")
                                    # lhsT: parity-matched q projT slice
                                    if cq % 2 == pk:
                                        lhs = qpT[
                                            64 * pk : 64 * (pk + 1),
                                            cq // 2,
                                            128 * hl : 128 * (hl + 1),
                                        ]
                                    else:
                                        cq2 = (cq - 1) % 8
                                        lhs = qpTr[
                                            64 * pk : 64 * (pk + 1),
                                            cq2 // 2,
                                            128 * hl : 128 * (hl + 1),
                                        ]
                                    for i2 in range(2):
                                        rhs = kpT[
                                            64 * pk : 64 * (pk + 1),
                                            2 * i2 : 2 * (i2 + 1),
                                            256 * hl : 256 * (hl + 1),
                                        ]
                                        nc.tensor.matmul(
                                            pss[:, 512 * i2 : 512 * (i2 + 1)],
                                            lhs,
                                            rhs,
                                            start=True,
                                            stop=True,
                                        )
                                    # additive mask (blocked col order matches psum)
                                    nc.vector.tensor_tensor(
                                        out=sm_t[:, 1024 * pk : 1024 * (pk + 1)],
                                        in0=pss[:],
                                        in1=mts[u2][:, 1024 * pk : 1024 * (pk + 1)],
                                        op=add,
                                    )
                                    # exp: un-permute k order via strided out
                                    eview = e_t.rearrange("p (rk c) -> p c rk", c=8)[
                                        :, pk : 8 : 2, :
                                    ]
                                    nc.scalar.activation(
                                        eview,
                                        sm_t[
                                            :, 1024 * pk : 1024 * (pk + 1)
                                        ].rearrange("p (c rk) -> p c rk", rk=256),
                                        Exp,
                                        accum_out=dent[:, pk : pk + 1],
                                    )
                                nc.vector.tensor_tensor(
                                    out=dent[:, 2:3],
                                    in0=dent[:, 0:1],
                                    in1=dent[:, 1:2],
                                    op=add,
                                )
                                nc.vector.reciprocal(dent[:, 3:4], dent[:, 2:3])
                                nc.vector.tensor_scalar(
                                    out=e_t[:],
                                    in0=e_t[:],
                                    scalar1=dent[:, 3:4],
                                    scalar2=None,
                                    op0=mult,
                                )
                                nc.sync.dma_start(attn_o[h, cq::8, :], e_t[:])
                                # bf16 copy of normalized attn for the @v path
                                a_bf = abp.tile([128, L], bf16, tag="ab")
                                if u2 == 0:
                                    nc.scalar.copy(out=a_bf[:], in_=e_t[:])
                                else:
                                    nc.vector.tensor_copy(out=a_bf[:], in_=e_t[:])
                                # transposes of blocked views for the @v matmul
                                ebl = a_bf.rearrange("p (rk c) -> p c rk", c=8)
                                for a in range(4):
                                    pt = tps.tile([128, 512], bf16, tag="tb512")
                                    for w in range(4):
                                        j = 4 * a + w
                                        ck = CKS[j // 2]
                                        nc.tensor.transpose(
                                            pt[:, 128 * w : 128 * (w + 1)],
                                            ebl[
                                                :,
                                                ck,
                                                128 * (j % 2) : 128 * (j % 2) + 128,
                                            ],
                                            identb_sb[:],
                                        )
                                    cpo = aT[
                                        :, 4 * a : 4 * a + 4, 128 * u2 : 128 * (u2 + 1)
                                    ]
                                    cpi = pt.rearrange("p (j q) -> p j q", j=4)
                                    if a % 2 == 0:
                                        nc.scalar.copy(out=cpo, in_=cpi)
                                    else:
                                        nc.vector.tensor_copy(out=cpo, in_=cpi)
                            # attn @ v for (head, cq pair)
                            pvt = vtp.tile([DH, 256], f32, tag="pvt")
                            for j in range(16):
                                nc.tensor.matmul(
                                    pvt[:],
                                    v_sb[:, hl, j, :],
                                    aT[:, j, :],
                                    start=(j == 0),
                                    stop=(j == 15),
                                )
                            # outT[d', hl, 8*rq + 2*cqp + u2] <- pvt[d', 128u2+rq]
                            dstT = outT[:, hl, :].rearrange(
                                "p (rq c) -> p c rq", c=8
                            )[:, 2 * cqp : 2 * cqp + 2, :]
                            nc.scalar.copy(
                                out=r(dstT),
                                in_=pvt.rearrange("p (c rq) -> p c rq", rq=128),
                            )

                    # output projection for this group
                    for hl in range(4):
                        h = 4 * hg + hl
                        po_t = sps.tile([128, 1024], f32, tag="s")
                        po = po_t[:, 0:512]
                        for c in range(8):
                            nc.tensor.matmul(
                                po[:],
                                r(outT[:, hl, c::8]),
                                r(wo_sb[:, c, :]),
                                start=(c == 0),
                                stop=False,
                            )
                        nc.tensor.matmul(
                            po[:],
                            r(ones[0:1, 0:128]),
                            r(brow["bo"][0:1, :]),
                            start=False,
                            stop=True,
                        )
                        o_sb = ost.tile([128, D], f32, tag="ost")
                        nc.scalar.copy(out=o_sb[:], in_=po[:])
                        nc.sync.dma_start(out_o[128 * h : 128 * (h + 1), :], o_sb[:])

    nc.finalize()
    return nc


def _get_nc():
    if "nc" not in _CACHE:
        _CACHE["nc"] = _build_nc()
    return _CACHE["nc"]


def _host_prep(inputs):
    Q = np.asarray(inputs["Q"], dtype=np.float32)
    K = np.asarray(inputs["K"], dtype=np.float32)
    V = np.asarray(inputs["V"], dtype=np.float32)
    mask = np.asarray(inputs["mask"])
    Wq = np.asarray(inputs["Wq"], dtype=np.float32) / 32.0
    bqv = np.asarray(inputs["bq"], dtype=np.float32) / 32.0
    Wqr = np.roll(Wq, -64, axis=1)
    bqr = np.roll(bqv, -64)
    com = {
        "wq": np.ascontiguousarray(Wq),
        "wqr": np.ascontiguousarray(Wqr),
        "wk": np.ascontiguousarray(np.asarray(inputs["Wk"], dtype=np.float32)),
        "wv": np.ascontiguousarray(np.asarray(inputs["Wv"], dtype=np.float32)),
        "wo": np.ascontiguousarray(np.asarray(inputs["Wo"], dtype=np.float32)),
        "bq": np.ascontiguousarray(bqv.reshape(1, D)),
        "bqr": np.ascontiguousarray(bqr.reshape(1, D)),
        "bk": np.ascontiguousarray(np.asarray(inputs["bk"], dtype=np.float32).reshape(1, D)),
        "bv": np.ascontiguousarray(np.asarray(inputs["bv"], dtype=np.float32).reshape(1, D)),
        "bo": np.ascontiguousarray(np.asarray(inputs["bo"], dtype=np.float32).reshape(1, D)),
        "onesr": np.ones((1, D), np.float32),
        "identr": np.eye(128, dtype=np.float32),
        "identb": np.eye(128, dtype=np.float32).astype(__import__("ml_dtypes").bfloat16),
    }
    qrows = np.concatenate([np.arange(256 * h, 256 * h + 128) for h in range(H)])
    in_maps = []
    for c in range(NCORES):
        b, g = c // 2, c % 2
        mslice = mask[b, 0, 1024 * g : 1024 * (g + 1), :].astype(np.float32)
        mbias = (mslice - 1.0) * 1e9
        # permute k columns to the blocked order the scores psum produces
        mb2v = mbias.reshape(LQ, 256, 8)[:, :, CKS].transpose(0, 2, 1).reshape(LQ, L)
        in_maps.append(
            {
                "xqt": np.ascontiguousarray(Q[b][qrows + 128 * g].T),
                "xkt": np.ascontiguousarray(K[b].T),
                "xvt": np.ascontiguousarray(V[b].T),
                "mb2": np.ascontiguousarray(mb2v),
                **com,
            }
        )
    return in_maps


def kernel(**inputs):
    from concourse.bass_utils import run_bass_kernel_spmd

    nc = _get_nc()
    in_maps = _host_prep(inputs)
    res = run_bass_kernel_spmd(nc, in_maps, list(range(NCORES))).results

    out = np.empty((B, L, D), np.float32)
    attn = np.empty((B, H, L, L), np.float32)
    for c in range(NCORES):
        b, g = c // 2, c % 2
        attn[b, :, 1024 * g : 1024 * (g + 1), :] = res[c]["attn"]
        o = res[c]["out"]
        for h in range(H):
            out[b, 256 * h + 128 * g : 256 * h + 128 * g + 128, :] = o[
                128 * h : 128 * (h + 1)
            ]
    return out, attn


# revision 11
# speedup vs baseline: 3.2627x; 1.0114x over previous
"""Bass/Trainium2 kernel for nn_Attention_481036337480.

Sharding: 8 cores = 4 batches x 2 query-halves. Core c handles batch
b=c//2 and query rows l' in [1024g, 1024(g+1)) (g=c%2) for ALL 8 heads.
The reference's raw reshape (B,L,D)->(B,H,L,dh) means head h only
involves projection rows [256h, 256h+256), so every per-core output
slice (attn_dist rows, out rows) is disjoint -> no collectives.

Layout trick: the reshape interleave l' = 8*r + c means head-h q/k/v
transposed slices are NOT plain APs of proj^T. Instead of de-interleaving
(which would need cross-partition copies the engines can't do), scores
are computed directly from projT psum layout as (cq, ck) 64-contraction
sub-blocks; a host-rolled copy of Wq supplies lhsT at the opposite
partition base for parity-mismatched (cq, ck) pairs. The blocked k-order
is un-permuted for free inside the strided-output exp pass.

Softmax: additive mask bias (host: (m-1)*1e9, k-block-permuted cols),
exp on ScalarE with accum_out denominators, normalize on GPSIMD.
"""

import sys

sys.path.insert(0, "/opt/trn_rl_repo")

import numpy as np

B, L, D, H = 4, 2048, 512, 8
DH = D // H  # 64
LQ = 1024  # q rows per core
NCORES = 8
CKS = [0, 2, 4, 6, 1, 3, 5, 7]  # k-block order as produced in scores psum

_CACHE = {}


def _patch_tile_drain():
    """walrus CTRL instructions accept only 1 sem wait; Tile's final drain
    attaches one wait per still-open semaphore. Chunk across chained drains."""
    from concourse.tile import TileContext
    from concourse.vector_clock import ScopedClock
    from concourse import mybir

    if getattr(TileContext, "_drain_patched", False):
        return

    def _drain_and_barrier(self, tick_clock, wait_clock):
        nc = self.nc
        drain_inst = nc.sync.drain()
        wait_clock.add_sem_waits(
            drain_inst.ins, ScopedClock({None: tick_clock.global_clock})
        )
        si = drain_inst.ins.sync_info
        waits = list(si.on_wait or [])
        if len(waits) > 1:
            si.on_wait.clear()
            si.on_wait.append(waits[0])
            for w in waits[1:]:
                extra = nc.sync.drain()
                if extra.ins.sync_info is None:
                    extra.ins.sync_info = mybir.SyncInfo(on_wait=[], on_update=[])
                extra.ins.sync_info.on_wait.append(w)
        nc.all_engine_barrier()
        assert self.sems is not None
        popped = nc._tile_sem_poison_stack.pop()
        assert popped is self._sem_poison
        nc.clear_and_free_semaphores(list(self.sems.allocated().values()))
        nc.all_engine_barrier()

    TileContext._drain_and_barrier = _drain_and_barrier
    TileContext._drain_patched = True


def _build_nc():
    import concourse.bass as bass
    from concourse import bacc, mybir
    from concourse.tile import TileContext
    from concourse.masks import make_identity
    from contextlib import ExitStack

    _patch_tile_drain()

    f32 = mybir.dt.float32
    f32r = mybir.dt.float32r
    bf16 = mybir.dt.bfloat16
    Exp = mybir.ActivationFunctionType.Exp
    mult = mybir.AluOpType.mult
    add = mybir.AluOpType.add

    def r(ap):
        return ap.bitcast(f32r)

    nc = bacc.Bacc()
    xqt = nc.declare_dram_parameter("xqt", [D, LQ], f32, isOutput=False)
    xkt = nc.declare_dram_parameter("xkt", [D, L], f32, isOutput=False)
    xvt = nc.declare_dram_parameter("xvt", [D, L], f32, isOutput=False)
    mb2 = nc.declare_dram_parameter("mb2", [LQ, L], f32, isOutput=False)
    wq = nc.declare_dram_parameter("wq", [D, D], f32, isOutput=False)
    wqr = nc.declare_dram_parameter("wqr", [D, D], f32, isOutput=False)
    wk = nc.declare_dram_parameter("wk", [D, D], f32, isOutput=False)
    wv = nc.declare_dram_parameter("wv", [D, D], f32, isOutput=False)
    wo = nc.declare_dram_parameter("wo", [D, D], f32, isOutput=False)
    bq = nc.declare_dram_parameter("bq", [1, D], f32, isOutput=False)
    bqr = nc.declare_dram_parameter("bqr", [1, D], f32, isOutput=False)
    bk = nc.declare_dram_parameter("bk", [1, D], f32, isOutput=False)
    bv = nc.declare_dram_parameter("bv", [1, D], f32, isOutput=False)
    bo = nc.declare_dram_parameter("bo", [1, D], f32, isOutput=False)
    onesr = nc.declare_dram_parameter("onesr", [1, D], f32, isOutput=False)
    identr = nc.declare_dram_parameter("identr", [128, 128], f32, isOutput=False)
    identb = nc.declare_dram_parameter("identb", [128, 128], mybir.dt.bfloat16, isOutput=False)
    attn_o = nc.declare_dram_parameter("attn", [H, LQ, L], f32, isOutput=True)
    out_o = nc.declare_dram_parameter("out", [LQ, D], f32, isOutput=True)

    with TileContext(nc) as tc, ExitStack() as ctx:
        const = ctx.enter_context(tc.tile_pool(name="const", bufs=1))
        ident = const.tile([128, 128], f32)
        nc.sync.dma_start(r(ident[:]), r(identr[:]))
        identb_sb = const.tile([128, 128], bf16, tag="identb")
        nc.sync.dma_start(identb_sb[:], identb[:])
        ones = const.tile([1, D], f32)
        nc.sync.dma_start(r(ones[:]), r(onesr[:]))
        brow = {}
        for nm, t in [("bq", bq), ("bqr", bqr), ("bk", bk), ("bv", bv), ("bo", bo)]:
            rt = const.tile([1, D], f32, tag=f"brow_{nm}")
            nc.sync.dma_start(r(rt[:]), r(t[:]))
            brow[nm] = rt
        # wo_sb[dd, c, o] = Wo[64c+dd, o]
        wo_sb = const.tile([DH, 8, D], f32, tag="wo_sb")
        for c in range(8):
            nc.sync.dma_start(
                r(wo_sb[:, c, :]), r(wo[64 * c : 64 * (c + 1), :])
            )

        for hg in range(2):  # head group: heads 4*hg .. 4*hg+4
            with ExitStack() as gctx:
                pers = gctx.enter_context(
                    tc.tile_pool(name=f"pers{hg}", bufs=1)
                )
                # projT activations, partition = o%128, chunk = o//128
                qpT = pers.tile([128, 4, 512], bf16, tag="qpT")
                qpTr = pers.tile([128, 4, 512], bf16, tag="qpTr")
                kpT = pers.tile([128, 4, 1024], bf16, tag="kpT")
                v_sb = pers.tile([128, 4, 16, DH], bf16, tag="v")
                outT = pers.tile([DH, 4, 1024], f32, tag="outT")

                # ---------- projections for this group ----------
                with (
                    tc.tile_pool(name=f"wp{hg}", bufs=1) as wpool,
                    tc.tile_pool(name=f"px{hg}", bufs=3) as projx,
                    tc.tile_pool(name=f"pp{hg}", bufs=1, space="PSUM") as projps,
                    tc.tile_pool(name=f"pv{hg}", bufs=2, space="PSUM") as vtps,
                ):
                    w_sb = {}
                    for nm, wt in [("wq", wq), ("wqr", wqr), ("wk", wk), ("wv", wv)]:
                        ws = wpool.tile([128, 4, D], f32, tag=f"w_{nm}")
                        for j in range(4):
                            nc.sync.dma_start(
                                r(ws[:, j, :]), r(wt[128 * j : 128 * (j + 1), :])
                            )
                        w_sb[nm] = ws
                    vpT = wpool.tile([128, 4, 1024], f32, tag="vpT")

                    def proj(xt, col0, nblk, wnm, bnm, dst):
                        for Bk in range(nblk):
                            xs = projx.tile([128, 4, 512], f32, tag="xs")
                            for j in range(4):
                                nc.sync.dma_start(
                                    r(xs[:, j, :]),
                                    r(
                                        xt[
                                            128 * j : 128 * (j + 1),
                                            col0 + 512 * Bk : col0 + 512 * (Bk + 1),
                                        ]
                                    ),
                                )
                            ps = projps.tile([128, 4, 512], f32, tag="pp")
                            for m in range(4):
                                for j in range(4):
                                    nc.tensor.matmul(
                                        ps[:, m, :],
                                        r(w_sb[wnm][:, j, 128 * m : 128 * (m + 1)]),
                                        r(xs[:, j, :]),
                                        start=(j == 0),
                                        stop=False,
                                    )
                                nc.tensor.matmul(
                                    ps[:, m, :],
                                    r(brow[bnm][0:1, 128 * m : 128 * (m + 1)]),
                                    r(ones[0:1, 0:512]),
                                    start=False,
                                    stop=True,
                                )
                            dv = dst[:, :, 512 * Bk : 512 * (Bk + 1)]
                            if dst.dtype != bf16:
                                dv = r(dv)
                            if Bk % 2 == 0:
                                nc.vector.tensor_copy(out=dv, in_=ps[:])
                            else:
                                nc.scalar.copy(out=dv, in_=ps[:])

                    proj(xqt, 512 * hg, 1, "wq", "bq", qpT)
                    proj(xqt, 512 * hg, 1, "wqr", "bqr", qpTr)
                    proj(xkt, 1024 * hg, 2, "wk", "bk", kpT)
                    proj(xvt, 1024 * hg, 2, "wv", "bv", vpT)

                    # v: transpose vpT blocks -> v_sb rows (k' on partitions, blocked)
                    for hl in range(4):
                        for a in range(2):  # j = 8*a + w
                            pv = vtps.tile([128, 512], f32, tag="pv")
                            for w in range(8):
                                j = 8 * a + w
                                ck = CKS[j // 2]
                                pk = ck % 2
                                nc.tensor.transpose(
                                    r(pv[:, 64 * w : 64 * (w + 1)]),
                                    r(
                                        vpT[
                                            64 * pk : 64 * (pk + 1),
                                            ck // 2,
                                            256 * hl + 128 * (j % 2) : 256 * hl
                                            + 128 * (j % 2)
                                            + 128,
                                        ]
                                    ),
                                    r(
                                        ident[
                                            64 * pk : 64 * (pk + 1),
                                            64 * pk : 64 * (pk + 1),
                                        ]
                                    ),
                                )
                            nc.scalar.copy(
                                out=v_sb[:, hl, 8 * a : 8 * a + 8, :],
                                in_=pv.rearrange("p (j d) -> p j d", j=8),
                            )

                # ---------- attention for this group ----------
                with (
                    tc.tile_pool(name=f"mt{hg}", bufs=4) as mtp,
                    tc.tile_pool(name=f"sm{hg}", bufs=2) as smp,
                    tc.tile_pool(name=f"ep{hg}", bufs=3) as epool,
                    tc.tile_pool(name=f"ab{hg}", bufs=2) as abp,
                    tc.tile_pool(name=f"aT{hg}", bufs=2) as aTp,
                    tc.tile_pool(name=f"dn{hg}", bufs=6) as dnp,
                    tc.tile_pool(name=f"os{hg}", bufs=2) as ost,
                    tc.tile_pool(name=f"s{hg}", bufs=2, space="PSUM") as sps,
                    tc.tile_pool(name=f"t{hg}", bufs=2, space="PSUM") as tps,
                    tc.tile_pool(name=f"vt{hg}", bufs=2, space="PSUM") as vtp,
                ):
                    for cqg in range(2):
                        mts = []
                        for u2 in range(4):
                            cq = 4 * cqg + u2
                            mtile = mtp.tile([128, L], f32, tag="mt")
                            nc.sync.dma_start(mtile[:], mb2[cq::8, :])
                            mts.append(mtile)
                        for hl in range(4):
                            h = 4 * hg + hl
                            aT = aTp.tile([128, 16, 512], bf16, tag="aT")
                            for u2 in range(4):
                                cq = 4 * cqg + u2
                                e_t = epool.tile([128, L], f32, tag="e")
                                sm_t = smp.tile([128, L], f32, tag="sm")
                                dent = dnp.tile([128, 4], f32, tag="den")
                                for pk in range(2):  # rhs partition half (ck parity)
                                    pss = sps.tile([128, 1024], f32, tag="s")
                                    # lhsT: parity-matched q projT slice
                                    if cq % 2 == pk:
                                        lhs = qpT[
                                            64 * pk : 64 * (pk + 1),
                                            cq // 2,
                                            128 * hl : 128 * (hl + 1),
                                        ]
                                    else:
                                        cq2 = (cq - 1) % 8
                                        lhs = qpTr[
                                            64 * pk : 64 * (pk + 1),
                                            cq2 // 2,
                                            128 * hl : 128 * (hl + 1),
                                        ]
                                    for i2 in range(2):
                                        rhs = kpT[
                                            64 * pk : 64 * (pk + 1),
                                            2 * i2 : 2 * (i2 + 1),
                                            256 * hl : 256 * (hl + 1),
                                        ]
                                        nc.tensor.matmul(
                                            pss[:, 512 * i2 : 512 * (i2 + 1)],
                                            lhs,
                                            rhs,
                                            start=True,
                                            stop=True,
                                        )
                                    # additive mask (blocked col order matches psum)
                                    nc.vector.tensor_tensor(
                                        out=sm_t[:, 1024 * pk : 1024 * (pk + 1)],
                                        in0=pss[:],
                                        in1=mts[u2][:, 1024 * pk : 1024 * (pk + 1)],
                                        op=add,
                                    )
                                    # exp: un-permute k order via strided out
                                    eview = e_t.rearrange("p (rk c) -> p c rk", c=8)[
                                        :, pk : 8 : 2, :
                                    ]
                                    nc.scalar.activation(
                                        eview,
                                        sm_t[
                                            :, 1024 * pk : 1024 * (pk + 1)
                                        ].rearrange("p (c rk) -> p c rk", rk=256),
                                        Exp,
                                        accum_out=dent[:, pk : pk + 1],
                                    )
                                nc.vector.tensor_tensor(
                                    out=dent[:, 2:3],
                                    in0=dent[:, 0:1],
                                    in1=dent[:, 1:2],
                                    op=add,
                                )
                                nc.vector.reciprocal(dent[:, 3:4], dent[:, 2:3])
                                nc.vector.tensor_scalar(
                                    out=e_t[:],
                                    in0=e_t[:],
                                    scalar1=dent[:, 3:4],
                                    scalar2=None,
                                    op0=mult,
                                )
                                nc.sync.dma_start(attn_o[h, cq::8, :], e_t[:])
                                # bf16 copy of normalized attn for the @v path
                                a_bf = abp.tile([128, L], bf16, tag="ab")
                                if u2 == 0:
                                    nc.scalar.copy(out=a_bf[:], in_=e_t[:])
                                else:
                                    nc.vector.tensor_copy(out=a_bf[:], in_=e_t[:])
                                # transposes of blocked views for the @v matmul
                                ebl = a_bf.rearrange("p (rk c) -> p c rk", c=8)
                                for a in range(4):
                                    pt = tps.tile([128, 512], bf16, tag="tb512")
                                    for w in range(4):
                                        j = 4 * a + w
                                        ck = CKS[j // 2]
                                        nc.tensor.transpose(
                                            pt[:, 128 * w : 128 * (w + 1)],
                                            ebl[
                                                :,
                                                ck,
                                                128 * (j % 2) : 128 * (j % 2) + 128,
                                            ],
                                            identb_sb[:],
                                        )
                                    cpo = aT[
                                        :, 4 * a : 4 * a + 4, 128 * u2 : 128 * (u2 + 1)
                                    ]  # u2 in 0..4 -> cols 0..512
                                    cpi = pt.rearrange("p (j q) -> p j q", j=4)
                                    if a % 2 == 0:
                                        nc.scalar.copy(out=cpo, in_=cpi)
                                    else:
                                        nc.vector.tensor_copy(out=cpo, in_=cpi)
                            # attn @ v for (head, cq quad)
                            pvt = vtp.tile([DH, 512], f32, tag="pvt")
                            for j in range(16):
                                nc.tensor.matmul(
                                    pvt[:],
                                    v_sb[:, hl, j, :],
                                    aT[:, j, :],
                                    start=(j == 0),
                                    stop=(j == 15),
                                )
                            # outT[d', hl, 8*rq + 4*cqg + u2] <- pvt[d', 128u2+rq]
                            dstT = outT[:, hl, :].rearrange(
                                "p (rq c) -> p c rq", c=8
                            )[:, 4 * cqg : 4 * cqg + 4, :]
                            nc.scalar.copy(
                                out=r(dstT),
                                in_=pvt.rearrange("p (c rq) -> p c rq", rq=128),
                            )

                    # output projection for this group
                    for hl in range(4):
                        h = 4 * hg + hl
                        po_t = sps.tile([128, 1024], f32, tag="s")
                        po = po_t[:, 0:512]
                        for c in range(8):
                            nc.tensor.matmul(
                                po[:],
                                r(outT[:, hl, c::8]),
                                r(wo_sb[:, c, :]),
                                start=(c == 0),
                                stop=False,
                            )
                        nc.tensor.matmul(
                            po[:],
                            r(ones[0:1, 0:128]),
                            r(brow["bo"][0:1, :]),
                            start=False,
                            stop=True,
                        )
                        o_sb = ost.tile([128, D], f32, tag="ost")
                        nc.scalar.copy(out=o_sb[:], in_=po[:])
                        nc.sync.dma_start(out_o[128 * h : 128 * (h + 1), :], o_sb[:])

    nc.finalize()
    return nc


def _get_nc():
    if "nc" not in _CACHE:
        _CACHE["nc"] = _build_nc()
    return _CACHE["nc"]


def _host_prep(inputs):
    Q = np.asarray(inputs["Q"], dtype=np.float32)
    K = np.asarray(inputs["K"], dtype=np.float32)
    V = np.asarray(inputs["V"], dtype=np.float32)
    mask = np.asarray(inputs["mask"])
    Wq = np.asarray(inputs["Wq"], dtype=np.float32) / 32.0
    bqv = np.asarray(inputs["bq"], dtype=np.float32) / 32.0
    Wqr = np.roll(Wq, -64, axis=1)
    bqr = np.roll(bqv, -64)
    com = {
        "wq": np.ascontiguousarray(Wq),
        "wqr": np.ascontiguousarray(Wqr),
        "wk": np.ascontiguousarray(np.asarray(inputs["Wk"], dtype=np.float32)),
        "wv": np.ascontiguousarray(np.asarray(inputs["Wv"], dtype=np.float32)),
        "wo": np.ascontiguousarray(np.asarray(inputs["Wo"], dtype=np.float32)),
        "bq": np.ascontiguousarray(bqv.reshape(1, D)),
        "bqr": np.ascontiguousarray(bqr.reshape(1, D)),
        "bk": np.ascontiguousarray(np.asarray(inputs["bk"], dtype=np.float32).reshape(1, D)),
        "bv": np.ascontiguousarray(np.asarray(inputs["bv"], dtype=np.float32).reshape(1, D)),
        "bo": np.ascontiguousarray(np.asarray(inputs["bo"], dtype=np.float32).reshape(1, D)),
        "onesr": np.ones((1, D), np.float32),
        "identr": np.eye(128, dtype=np.float32),
        "identb": np.eye(128, dtype=np.float32).astype(__import__("ml_dtypes").bfloat16),
    }
    qrows = np.concatenate([np.arange(256 * h, 256 * h + 128) for h in range(H)])
    in_maps = []
    for c in range(NCORES):
        b, g = c // 2, c % 2
        mslice = mask[b, 0, 1024 * g : 1024 * (g + 1), :].astype(np.float32)
        mbias = (mslice - 1.0) * 1e9
        # permute k columns to the blocked order the scores psum produces
        mb2v = mbias.reshape(LQ, 256, 8)[:, :, CKS].transpose(0, 2, 1).reshape(LQ, L)
        in_maps.append(
            {
                "xqt": np.ascontiguousarray(Q[b][qrows + 128 * g].T),
                "xkt": np.ascontiguousarray(K[b].T),
                "xvt": np.ascontiguousarray(V[b].T),
                "mb2": np.ascontiguousarray(mb2v),
                **com,
            }
        )
    return in_maps


def kernel(**inputs):
    from concourse.bass_utils import run_bass_kernel_spmd

    nc = _get_nc()
    in_maps = _host_prep(inputs)
    res = run_bass_kernel_spmd(nc, in_maps, list(range(NCORES))).results

    out = np.empty((B, L, D), np.float32)
    attn = np.empty((B, H, L, L), np.float32)
    for c in range(NCORES):
        b, g = c // 2, c % 2
        attn[b, :, 1024 * g : 1024 * (g + 1), :] = res[c]["attn"]
        o = res[c]["out"]
        for h in range(H):
            out[b, 256 * h + 128 * g : 256 * h + 128 * g + 128, :] = o[
                128 * h : 128 * (h + 1)
            ]
    return out, attn


# revision 12
# speedup vs baseline: 3.2883x; 1.0078x over previous
"""Bass/Trainium2 kernel for nn_Attention_481036337480.

Sharding: 8 cores = 4 batches x 2 query-halves. Core c handles batch
b=c//2 and query rows l' in [1024g, 1024(g+1)) (g=c%2) for ALL 8 heads.
The reference's raw reshape (B,L,D)->(B,H,L,dh) means head h only
involves projection rows [256h, 256h+256), so every per-core output
slice (attn_dist rows, out rows) is disjoint -> no collectives.

Layout trick: the reshape interleave l' = 8*r + c means head-h q/k/v
transposed slices are NOT plain APs of proj^T. Instead of de-interleaving
(which would need cross-partition copies the engines can't do), scores
are computed directly from projT psum layout as (cq, ck) 64-contraction
sub-blocks; a host-rolled copy of Wq supplies lhsT at the opposite
partition base for parity-mismatched (cq, ck) pairs. The blocked k-order
is un-permuted for free inside the strided-output exp pass.

Softmax: additive mask bias (host: (m-1)*1e9, k-block-permuted cols),
exp on ScalarE with accum_out denominators, normalize on GPSIMD.
"""

import sys

sys.path.insert(0, "/opt/trn_rl_repo")

import numpy as np

B, L, D, H = 4, 2048, 512, 8
DH = D // H  # 64
LQ = 1024  # q rows per core
NCORES = 8
CKS = [0, 2, 4, 6, 1, 3, 5, 7]  # k-block order as produced in scores psum

_CACHE = {}


def _patch_tile_drain():
    """walrus CTRL instructions accept only 1 sem wait; Tile's final drain
    attaches one wait per still-open semaphore. Chunk across chained drains."""
    from concourse.tile import TileContext
    from concourse.vector_clock import ScopedClock
    from concourse import mybir

    if getattr(TileContext, "_drain_patched", False):
        return

    def _drain_and_barrier(self, tick_clock, wait_clock):
        nc = self.nc
        drain_inst = nc.sync.drain()
        wait_clock.add_sem_waits(
            drain_inst.ins, ScopedClock({None: tick_clock.global_clock})
        )
        si = drain_inst.ins.sync_info
        waits = list(si.on_wait or [])
        if len(waits) > 1:
            si.on_wait.clear()
            si.on_wait.append(waits[0])
            for w in waits[1:]:
                extra = nc.sync.drain()
                if extra.ins.sync_info is None:
                    extra.ins.sync_info = mybir.SyncInfo(on_wait=[], on_update=[])
                extra.ins.sync_info.on_wait.append(w)
        nc.all_engine_barrier()
        assert self.sems is not None
        popped = nc._tile_sem_poison_stack.pop()
        assert popped is self._sem_poison
        nc.clear_and_free_semaphores(list(self.sems.allocated().values()))
        nc.all_engine_barrier()

    TileContext._drain_and_barrier = _drain_and_barrier
    TileContext._drain_patched = True


def _build_nc():
    import concourse.bass as bass
    from concourse import bacc, mybir
    from concourse.tile import TileContext
    from concourse.masks import make_identity
    from contextlib import ExitStack

    _patch_tile_drain()

    f32 = mybir.dt.float32
    f32r = mybir.dt.float32r
    bf16 = mybir.dt.bfloat16
    Exp = mybir.ActivationFunctionType.Exp
    mult = mybir.AluOpType.mult
    add = mybir.AluOpType.add

    def r(ap):
        return ap.bitcast(f32r)

    nc = bacc.Bacc()
    xqt = nc.declare_dram_parameter("xqt", [D, LQ], f32, isOutput=False)
    xkt = nc.declare_dram_parameter("xkt", [D, L], f32, isOutput=False)
    xvt = nc.declare_dram_parameter("xvt", [D, L], f32, isOutput=False)
    mb2 = nc.declare_dram_parameter("mb2", [LQ, L], f32, isOutput=False)
    wq = nc.declare_dram_parameter("wq", [D, D], f32, isOutput=False)
    wqr = nc.declare_dram_parameter("wqr", [D, D], f32, isOutput=False)
    wk = nc.declare_dram_parameter("wk", [D, D], f32, isOutput=False)
    wv = nc.declare_dram_parameter("wv", [D, D], f32, isOutput=False)
    wo = nc.declare_dram_parameter("wo", [D, D], f32, isOutput=False)
    bq = nc.declare_dram_parameter("bq", [1, D], f32, isOutput=False)
    bqr = nc.declare_dram_parameter("bqr", [1, D], f32, isOutput=False)
    bk = nc.declare_dram_parameter("bk", [1, D], f32, isOutput=False)
    bv = nc.declare_dram_parameter("bv", [1, D], f32, isOutput=False)
    bo = nc.declare_dram_parameter("bo", [1, D], f32, isOutput=False)
    onesr = nc.declare_dram_parameter("onesr", [1, D], f32, isOutput=False)
    identr = nc.declare_dram_parameter("identr", [128, 128], f32, isOutput=False)
    identb = nc.declare_dram_parameter("identb", [128, 128], mybir.dt.bfloat16, isOutput=False)
    attn_o = nc.declare_dram_parameter("attn", [H, LQ, L], f32, isOutput=True)
    out_o = nc.declare_dram_parameter("out", [LQ, D], f32, isOutput=True)

    with TileContext(nc) as tc, ExitStack() as ctx:
        const = ctx.enter_context(tc.tile_pool(name="const", bufs=1))
        ident = const.tile([128, 128], f32)
        nc.sync.dma_start(r(ident[:]), r(identr[:]))
        identb_sb = const.tile([128, 128], bf16, tag="identb")
        nc.sync.dma_start(identb_sb[:], identb[:])
        ones = const.tile([1, D], f32)
        nc.sync.dma_start(r(ones[:]), r(onesr[:]))
        brow = {}
        for nm, t in [("bq", bq), ("bqr", bqr), ("bk", bk), ("bv", bv), ("bo", bo)]:
            rt = const.tile([1, D], f32, tag=f"brow_{nm}")
            nc.sync.dma_start(r(rt[:]), r(t[:]))
            brow[nm] = rt
        # wo_sb[dd, c, o] = Wo[64c+dd, o]
        wo_sb = const.tile([DH, 8, D], f32, tag="wo_sb")
        for c in range(8):
            nc.sync.dma_start(
                r(wo_sb[:, c, :]), r(wo[64 * c : 64 * (c + 1), :])
            )

        for hg in range(2):  # head group: heads 4*hg .. 4*hg+4
            with ExitStack() as gctx:
                pers = gctx.enter_context(
                    tc.tile_pool(name=f"pers{hg}", bufs=1)
                )
                # projT activations, partition = o%128, chunk = o//128
                qpT = pers.tile([128, 4, 512], bf16, tag="qpT")
                qpTr = pers.tile([128, 4, 512], bf16, tag="qpTr")
                kpT = pers.tile([128, 4, 1024], bf16, tag="kpT")
                v_sb = pers.tile([128, 4, 16, DH], bf16, tag="v")
                outT = pers.tile([DH, 4, 1024], f32, tag="outT")

                # ---------- projections for this group ----------
                with (
                    tc.tile_pool(name=f"wp{hg}", bufs=1) as wpool,
                    tc.tile_pool(name=f"px{hg}", bufs=3) as projx,
                    tc.tile_pool(name=f"pp{hg}", bufs=1, space="PSUM") as projps,
                    tc.tile_pool(name=f"pv{hg}", bufs=2, space="PSUM") as vtps,
                ):
                    w_sb = {}
                    for nm, wt in [("wq", wq), ("wqr", wqr), ("wk", wk), ("wv", wv)]:
                        ws = wpool.tile([128, 4, D], f32, tag=f"w_{nm}")
                        for j in range(4):
                            nc.sync.dma_start(
                                r(ws[:, j, :]), r(wt[128 * j : 128 * (j + 1), :])
                            )
                        w_sb[nm] = ws
                    vpT = wpool.tile([128, 4, 1024], f32, tag="vpT")

                    def proj(xt, col0, nblk, wnm, bnm, dst):
                        for Bk in range(nblk):
                            xs = projx.tile([128, 4, 512], f32, tag="xs")
                            for j in range(4):
                                nc.sync.dma_start(
                                    r(xs[:, j, :]),
                                    r(
                                        xt[
                                            128 * j : 128 * (j + 1),
                                            col0 + 512 * Bk : col0 + 512 * (Bk + 1),
                                        ]
                                    ),
                                )
                            ps = projps.tile([128, 4, 512], f32, tag="pp")
                            for m in range(4):
                                for j in range(4):
                                    nc.tensor.matmul(
                                        ps[:, m, :],
                                        r(w_sb[wnm][:, j, 128 * m : 128 * (m + 1)]),
                                        r(xs[:, j, :]),
                                        start=(j == 0),
                                        stop=False,
                                    )
                                nc.tensor.matmul(
                                    ps[:, m, :],
                                    r(brow[bnm][0:1, 128 * m : 128 * (m + 1)]),
                                    r(ones[0:1, 0:512]),
                                    start=False,
                                    stop=True,
                                )
                            dv = dst[:, :, 512 * Bk : 512 * (Bk + 1)]
                            if dst.dtype != bf16:
                                dv = r(dv)
                            if Bk % 2 == 0:
                                nc.vector.tensor_copy(out=dv, in_=ps[:])
                            else:
                                nc.scalar.copy(out=dv, in_=ps[:])

                    proj(xqt, 512 * hg, 1, "wq", "bq", qpT)
                    proj(xqt, 512 * hg, 1, "wqr", "bqr", qpTr)
                    proj(xkt, 1024 * hg, 2, "wk", "bk", kpT)
                    proj(xvt, 1024 * hg, 2, "wv", "bv", vpT)

                    # v: transpose vpT blocks -> v_sb rows (k' on partitions, blocked)
                    for hl in range(4):
                        for a in range(2):  # j = 8*a + w
                            pv = vtps.tile([128, 512], f32, tag="pv")
                            for w in range(8):
                                j = 8 * a + w
                                ck = CKS[j // 2]
                                pk = ck % 2
                                nc.tensor.transpose(
                                    r(pv[:, 64 * w : 64 * (w + 1)]),
                                    r(
                                        vpT[
                                            64 * pk : 64 * (pk + 1),
                                            ck // 2,
                                            256 * hl + 128 * (j % 2) : 256 * hl
                                            + 128 * (j % 2)
                                            + 128,
                                        ]
                                    ),
                                    r(
                                        ident[
                                            64 * pk : 64 * (pk + 1),
                                            64 * pk : 64 * (pk + 1),
                                        ]
                                    ),
                                )
                            nc.scalar.copy(
                                out=v_sb[:, hl, 8 * a : 8 * a + 8, :],
                                in_=pv.rearrange("p (j d) -> p j d", j=8),
                            )

                # ---------- attention for this group ----------
                with (
                    tc.tile_pool(name=f"mt{hg}", bufs=4) as mtp,
                    tc.tile_pool(name=f"sm{hg}", bufs=3) as smp,
                    tc.tile_pool(name=f"ep{hg}", bufs=4) as epool,
                    tc.tile_pool(name=f"ab{hg}", bufs=3) as abp,
                    tc.tile_pool(name=f"aT{hg}", bufs=2) as aTp,
                    tc.tile_pool(name=f"dn{hg}", bufs=6) as dnp,
                    tc.tile_pool(name=f"os{hg}", bufs=2) as ost,
                    tc.tile_pool(name=f"s{hg}", bufs=2, space="PSUM") as sps,
                    tc.tile_pool(name=f"t{hg}", bufs=2, space="PSUM") as tps,
                    tc.tile_pool(name=f"vt{hg}", bufs=2, space="PSUM") as vtp,
                ):
                    for cqg in range(2):
                        mts = []
                        for u2 in range(4):
                            cq = 4 * cqg + u2
                            mtile = mtp.tile([128, L], f32, tag="mt")
                            nc.sync.dma_start(mtile[:], mb2[cq::8, :])
                            mts.append(mtile)
                        for hl in range(4):
                            h = 4 * hg + hl
                            aT = aTp.tile([128, 16, 512], bf16, tag="aT")
                            for u2 in range(4):
                                cq = 4 * cqg + u2
                                e_t = epool.tile([128, L], f32, tag="e")
                                sm_t = smp.tile([128, L], f32, tag="sm")
                                dent = dnp.tile([128, 4], f32, tag="den")
                                for pk in range(2):  # rhs partition half (ck parity)
                                    pss = sps.tile([128, 1024], f32, tag="s")
                                    # lhsT: parity-matched q projT slice
                                    if cq % 2 == pk:
                                        lhs = qpT[
                                            64 * pk : 64 * (pk + 1),
                                            cq // 2,
                                            128 * hl : 128 * (hl + 1),
                                        ]
                                    else:
                                        cq2 = (cq - 1) % 8
                                        lhs = qpTr[
                                            64 * pk : 64 * (pk + 1),
                                            cq2 // 2,
                                            128 * hl : 128 * (hl + 1),
                                        ]
                                    for i2 in range(2):
                                        rhs = kpT[
                                            64 * pk : 64 * (pk + 1),
                                            2 * i2 : 2 * (i2 + 1),
                                            256 * hl : 256 * (hl + 1),
                                        ]
                                        nc.tensor.matmul(
                                            pss[:, 512 * i2 : 512 * (i2 + 1)],
                                            lhs,
                                            rhs,
                                            start=True,
                                            stop=True,
                                        )
                                    # additive mask (blocked col order matches psum)
                                    nc.vector.tensor_tensor(
                                        out=sm_t[:, 1024 * pk : 1024 * (pk + 1)],
                                        in0=pss[:],
                                        in1=mts[u2][:, 1024 * pk : 1024 * (pk + 1)],
                                        op=add,
                                    )
                                    # exp: un-permute k order via strided out
                                    eview = e_t.rearrange("p (rk c) -> p c rk", c=8)[
                                        :, pk : 8 : 2, :
                                    ]
                                    nc.scalar.activation(
                                        eview,
                                        sm_t[
                                            :, 1024 * pk : 1024 * (pk + 1)
                                        ].rearrange("p (c rk) -> p c rk", rk=256),
                                        Exp,
                                        accum_out=dent[:, pk : pk + 1],
                                    )
                                nc.vector.tensor_tensor(
                                    out=dent[:, 2:3],
                                    in0=dent[:, 0:1],
                                    in1=dent[:, 1:2],
                                    op=add,
                                )
                                nc.vector.reciprocal(dent[:, 3:4], dent[:, 2:3])
                                nc.vector.tensor_scalar(
                                    out=e_t[:],
                                    in0=e_t[:],
                                    scalar1=dent[:, 3:4],
                                    scalar2=None,
                                    op0=mult,
                                )
                                nc.sync.dma_start(attn_o[h, cq::8, :], e_t[:])
                                # bf16 copy of normalized attn for the @v path
                                a_bf = abp.tile([128, L], bf16, tag="ab")
                                if u2 == 0:
                                    nc.scalar.copy(out=a_bf[:], in_=e_t[:])
                                else:
                                    nc.vector.tensor_copy(out=a_bf[:], in_=e_t[:])
                                # transposes of blocked views for the @v matmul
                                ebl = a_bf.rearrange("p (rk c) -> p c rk", c=8)
                                for a in range(4):
                                    pt = tps.tile([128, 512], bf16, tag="tb512")
                                    for w in range(4):
                                        j = 4 * a + w
                                        ck = CKS[j // 2]
                                        nc.tensor.transpose(
                                            pt[:, 128 * w : 128 * (w + 1)],
                                            ebl[
                                                :,
                                                ck,
                                                128 * (j % 2) : 128 * (j % 2) + 128,
                                            ],
                                            identb_sb[:],
                                        )
                                    cpo = aT[
                                        :, 4 * a : 4 * a + 4, 128 * u2 : 128 * (u2 + 1)
                                    ]  # u2 in 0..4 -> cols 0..512
                                    cpi = pt.rearrange("p (j q) -> p j q", j=4)
                                    if a % 2 == 0:
                                        nc.scalar.copy(out=cpo, in_=cpi)
                                    else:
                                        nc.vector.tensor_copy(out=cpo, in_=cpi)
                            # attn @ v for (head, cq quad)
                            pvt = vtp.tile([DH, 512], f32, tag="pvt")
                            for j in range(16):
                                nc.tensor.matmul(
                                    pvt[:],
                                    v_sb[:, hl, j, :],
                                    aT[:, j, :],
                                    start=(j == 0),
                                    stop=(j == 15),
                                )
                            # outT[d', hl, 8*rq + 4*cqg + u2] <- pvt[d', 128u2+rq]
                            dstT = outT[:, hl, :].rearrange(
                                "p (rq c) -> p c rq", c=8
                            )[:, 4 * cqg : 4 * cqg + 4, :]
                            nc.scalar.copy(
                                out=r(dstT),
                                in_=pvt.rearrange("p (c rq) -> p c rq", rq=128),
                            )

                    # output projection for this group
                    for hl in range(4):
                        h = 4 * hg + hl
                        po_t = sps.tile([128, 1024], f32, tag="s")
                        po = po_t[:, 0:512]
                        for c in range(8):
                            nc.tensor.matmul(
                                po[:],
                                r(outT[:, hl, c::8]),
                                r(wo_sb[:, c, :]),
                                start=(c == 0),
                                stop=False,
                            )
                        nc.tensor.matmul(
                            po[:],
                            r(ones[0:1, 0:128]),
                            r(brow["bo"][0:1, :]),
                            start=False,
                            stop=True,
                        )
                        o_sb = ost.tile([128, D], f32, tag="ost")
                        nc.scalar.copy(out=o_sb[:], in_=po[:])
                        nc.sync.dma_start(out_o[128 * h : 128 * (h + 1), :], o_sb[:])

    nc.finalize()
    return nc


def _get_nc():
    if "nc" not in _CACHE:
        _CACHE["nc"] = _build_nc()
    return _CACHE["nc"]


def _host_prep(inputs):
    Q = np.asarray(inputs["Q"], dtype=np.float32)
    K = np.asarray(inputs["K"], dtype=np.float32)
    V = np.asarray(inputs["V"], dtype=np.float32)
    mask = np.asarray(inputs["mask"])
    Wq = np.asarray(inputs["Wq"], dtype=np.float32) / 32.0
    bqv = np.asarray(inputs["bq"], dtype=np.float32) / 32.0
    Wqr = np.roll(Wq, -64, axis=1)
    bqr = np.roll(bqv, -64)
    com = {
        "wq": np.ascontiguousarray(Wq),
        "wqr": np.ascontiguousarray(Wqr),
        "wk": np.ascontiguousarray(np.asarray(inputs["Wk"], dtype=np.float32)),
        "wv": np.ascontiguousarray(np.asarray(inputs["Wv"], dtype=np.float32)),
        "wo": np.ascontiguousarray(np.asarray(inputs["Wo"], dtype=np.float32)),
        "bq": np.ascontiguousarray(bqv.reshape(1, D)),
        "bqr": np.ascontiguousarray(bqr.reshape(1, D)),
        "bk": np.ascontiguousarray(np.asarray(inputs["bk"], dtype=np.float32).reshape(1, D)),
        "bv": np.ascontiguousarray(np.asarray(inputs["bv"], dtype=np.float32).reshape(1, D)),
        "bo": np.ascontiguousarray(np.asarray(inputs["bo"], dtype=np.float32).reshape(1, D)),
        "onesr": np.ones((1, D), np.float32),
        "identr": np.eye(128, dtype=np.float32),
        "identb": np.eye(128, dtype=np.float32).astype(__import__("ml_dtypes").bfloat16),
    }
    qrows = np.concatenate([np.arange(256 * h, 256 * h + 128) for h in range(H)])
    in_maps = []
    for c in range(NCORES):
        b, g = c // 2, c % 2
        mslice = mask[b, 0, 1024 * g : 1024 * (g + 1), :].astype(np.float32)
        mbias = (mslice - 1.0) * 1e9
        # permute k columns to the blocked order the scores psum produces
        mb2v = mbias.reshape(LQ, 256, 8)[:, :, CKS].transpose(0, 2, 1).reshape(LQ, L)
        in_maps.append(
            {
                "xqt": np.ascontiguousarray(Q[b][qrows + 128 * g].T),
                "xkt": np.ascontiguousarray(K[b].T),
                "xvt": np.ascontiguousarray(V[b].T),
                "mb2": np.ascontiguousarray(mb2v),
                **com,
            }
        )
    return in_maps


def kernel(**inputs):
    from concourse.bass_utils import run_bass_kernel_spmd

    nc = _get_nc()
    in_maps = _host_prep(inputs)
    res = run_bass_kernel_spmd(nc, in_maps, list(range(NCORES))).results

    out = np.empty((B, L, D), np.float32)
    attn = np.empty((B, H, L, L), np.float32)
    for c in range(NCORES):
        b, g = c // 2, c % 2
        attn[b, :, 1024 * g : 1024 * (g + 1), :] = res[c]["attn"]
        o = res[c]["out"]
        for h in range(H):
            out[b, 256 * h + 128 * g : 256 * h + 128 * g + 128, :] = o[
                128 * h : 128 * (h + 1)
            ]
    return out, attn


# revision 13
# speedup vs baseline: 3.3218x; 1.0102x over previous
"""Bass/Trainium2 kernel for nn_Attention_481036337480.

Sharding: 8 cores = 4 batches x 2 query-halves. Core c handles batch
b=c//2 and query rows l' in [1024g, 1024(g+1)) (g=c%2) for ALL 8 heads.
The reference's raw reshape (B,L,D)->(B,H,L,dh) means head h only
involves projection rows [256h, 256h+256), so every per-core output
slice (attn_dist rows, out rows) is disjoint -> no collectives.

Layout trick: the reshape interleave l' = 8*r + c means head-h q/k/v
transposed slices are NOT plain APs of proj^T. Instead of de-interleaving
(which would need cross-partition copies the engines can't do), scores
are computed directly from projT psum layout as (cq, ck) 64-contraction
sub-blocks; a host-rolled copy of Wq supplies lhsT at the opposite
partition base for parity-mismatched (cq, ck) pairs. The blocked k-order
is un-permuted for free inside the strided-output exp pass.

Softmax: additive mask bias (host: (m-1)*1e9, k-block-permuted cols),
exp on ScalarE with accum_out denominators, normalize on GPSIMD.
"""

import sys

sys.path.insert(0, "/opt/trn_rl_repo")

import numpy as np

B, L, D, H = 4, 2048, 512, 8
DH = D // H  # 64
LQ = 1024  # q rows per core
NCORES = 8
CKS = [0, 2, 4, 6, 1, 3, 5, 7]  # k-block order as produced in scores psum

_CACHE = {}


def _patch_tile_drain():
    """walrus CTRL instructions accept only 1 sem wait; Tile's final drain
    attaches one wait per still-open semaphore. Chunk across chained drains."""
    from concourse.tile import TileContext
    from concourse.vector_clock import ScopedClock
    from concourse import mybir

    if getattr(TileContext, "_drain_patched", False):
        return

    def _drain_and_barrier(self, tick_clock, wait_clock):
        nc = self.nc
        drain_inst = nc.sync.drain()
        wait_clock.add_sem_waits(
            drain_inst.ins, ScopedClock({None: tick_clock.global_clock})
        )
        si = drain_inst.ins.sync_info
        waits = list(si.on_wait or [])
        if len(waits) > 1:
            si.on_wait.clear()
            si.on_wait.append(waits[0])
            for w in waits[1:]:
                extra = nc.sync.drain()
                if extra.ins.sync_info is None:
                    extra.ins.sync_info = mybir.SyncInfo(on_wait=[], on_update=[])
                extra.ins.sync_info.on_wait.append(w)
        nc.all_engine_barrier()
        assert self.sems is not None
        popped = nc._tile_sem_poison_stack.pop()
        assert popped is self._sem_poison
        nc.clear_and_free_semaphores(list(self.sems.allocated().values()))
        nc.all_engine_barrier()

    TileContext._drain_and_barrier = _drain_and_barrier
    TileContext._drain_patched = True


def _build_nc():
    import concourse.bass as bass
    from concourse import bacc, mybir
    from concourse.tile import TileContext
    from concourse.masks import make_identity
    from contextlib import ExitStack

    _patch_tile_drain()

    f32 = mybir.dt.float32
    f32r = mybir.dt.float32r
    bf16 = mybir.dt.bfloat16
    Exp = mybir.ActivationFunctionType.Exp
    mult = mybir.AluOpType.mult
    add = mybir.AluOpType.add

    def r(ap):
        return ap.bitcast(f32r)

    nc = bacc.Bacc()
    xqt = nc.declare_dram_parameter("xqt", [D, LQ], f32, isOutput=False)
    xkt = nc.declare_dram_parameter("xkt", [D, L], f32, isOutput=False)
    xvt = nc.declare_dram_parameter("xvt", [D, L], f32, isOutput=False)
    mb2 = nc.declare_dram_parameter("mb2", [LQ, L], f32, isOutput=False)
    wq = nc.declare_dram_parameter("wq", [D, D], f32, isOutput=False)
    wqr = nc.declare_dram_parameter("wqr", [D, D], f32, isOutput=False)
    wk = nc.declare_dram_parameter("wk", [D, D], f32, isOutput=False)
    wv = nc.declare_dram_parameter("wv", [D, D], f32, isOutput=False)
    wo = nc.declare_dram_parameter("wo", [D, D], f32, isOutput=False)
    bq = nc.declare_dram_parameter("bq", [1, D], f32, isOutput=False)
    bqr = nc.declare_dram_parameter("bqr", [1, D], f32, isOutput=False)
    bk = nc.declare_dram_parameter("bk", [1, D], f32, isOutput=False)
    bv = nc.declare_dram_parameter("bv", [1, D], f32, isOutput=False)
    bo = nc.declare_dram_parameter("bo", [1, D], f32, isOutput=False)
    onesr = nc.declare_dram_parameter("onesr", [1, D], f32, isOutput=False)
    identr = nc.declare_dram_parameter("identr", [128, 128], f32, isOutput=False)
    identb = nc.declare_dram_parameter("identb", [128, 128], mybir.dt.bfloat16, isOutput=False)
    attn_o = nc.declare_dram_parameter("attn", [H, LQ, L], f32, isOutput=True)
    out_o = nc.declare_dram_parameter("out", [LQ, D], f32, isOutput=True)

    with TileContext(nc) as tc, ExitStack() as ctx:
        const = ctx.enter_context(tc.tile_pool(name="const", bufs=1))
        ident = const.tile([128, 128], f32)
        nc.sync.dma_start(r(ident[:]), r(identr[:]))
        identb_sb = const.tile([128, 128], bf16, tag="identb")
        nc.sync.dma_start(identb_sb[:], identb[:])
        ones = const.tile([1, D], f32)
        nc.sync.dma_start(r(ones[:]), r(onesr[:]))
        brow = {}
        for nm, t in [("bq", bq), ("bqr", bqr), ("bk", bk), ("bv", bv), ("bo", bo)]:
            rt = const.tile([1, D], f32, tag=f"brow_{nm}")
            nc.sync.dma_start(r(rt[:]), r(t[:]))
            brow[nm] = rt
        # wo_sb[dd, c, o] = Wo[64c+dd, o]
        wo_sb = const.tile([DH, 8, D], f32, tag="wo_sb")
        for c in range(8):
            nc.sync.dma_start(
                r(wo_sb[:, c, :]), r(wo[64 * c : 64 * (c + 1), :])
            )

        for hg in range(2):  # head group: heads 4*hg .. 4*hg+4
            with ExitStack() as gctx:
                pers = gctx.enter_context(
                    tc.tile_pool(name=f"pers{hg}", bufs=1)
                )
                # projT activations, partition = o%128, chunk = o//128
                qpT = pers.tile([128, 4, 512], bf16, tag="qpT")
                qpTr = pers.tile([128, 4, 512], bf16, tag="qpTr")
                kpT = pers.tile([128, 4, 1024], bf16, tag="kpT")
                v_sb = pers.tile([128, 4, 16, DH], bf16, tag="v")
                outT = pers.tile([DH, 4, 1024], f32, tag="outT")

                # ---------- projections for this group ----------
                with (
                    tc.tile_pool(name=f"wp{hg}", bufs=1) as wpool,
                    tc.tile_pool(name=f"px{hg}", bufs=3) as projx,
                    tc.tile_pool(name=f"pp{hg}", bufs=1, space="PSUM") as projps,
                    tc.tile_pool(name=f"pv{hg}", bufs=2, space="PSUM") as vtps,
                ):
                    w_sb = {}
                    for nm, wt in [("wq", wq), ("wqr", wqr), ("wk", wk), ("wv", wv)]:
                        ws = wpool.tile([128, 4, D], f32, tag=f"w_{nm}")
                        for j in range(4):
                            nc.sync.dma_start(
                                r(ws[:, j, :]), r(wt[128 * j : 128 * (j + 1), :])
                            )
                        w_sb[nm] = ws
                    vpT = wpool.tile([128, 4, 1024], f32, tag="vpT")

                    def proj(xt, col0, nblk, wnm, bnm, dst):
                        for Bk in range(nblk):
                            xs = projx.tile([128, 4, 512], f32, tag="xs")
                            for j in range(4):
                                nc.sync.dma_start(
                                    r(xs[:, j, :]),
                                    r(
                                        xt[
                                            128 * j : 128 * (j + 1),
                                            col0 + 512 * Bk : col0 + 512 * (Bk + 1),
                                        ]
                                    ),
                                )
                            ps = projps.tile([128, 4, 512], f32, tag="pp")
                            for m in range(4):
                                for j in range(4):
                                    nc.tensor.matmul(
                                        ps[:, m, :],
                                        r(w_sb[wnm][:, j, 128 * m : 128 * (m + 1)]),
                                        r(xs[:, j, :]),
                                        start=(j == 0),
                                        stop=False,
                                    )
                                nc.tensor.matmul(
                                    ps[:, m, :],
                                    r(brow[bnm][0:1, 128 * m : 128 * (m + 1)]),
                                    r(ones[0:1, 0:512]),
                                    start=False,
                                    stop=True,
                                )
                            dv = dst[:, :, 512 * Bk : 512 * (Bk + 1)]
                            if dst.dtype != bf16:
                                dv = r(dv)
                            if Bk % 2 == 0:
                                nc.vector.tensor_copy(out=dv, in_=ps[:])
                            else:
                                nc.scalar.copy(out=dv, in_=ps[:])

                    proj(xqt, 512 * hg, 1, "wq", "bq", qpT)
                    proj(xqt, 512 * hg, 1, "wqr", "bqr", qpTr)
                    proj(xkt, 1024 * hg, 2, "wk", "bk", kpT)
                    proj(xvt, 1024 * hg, 2, "wv", "bv", vpT)

                    # v: transpose vpT blocks -> v_sb rows (k' on partitions, blocked)
                    for hl in range(4):
                        for a in range(2):  # j = 8*a + w
                            pv = vtps.tile([128, 512], f32, tag="pv")
                            for w in range(8):
                                j = 8 * a + w
                                ck = CKS[j // 2]
                                pk = ck % 2
                                nc.tensor.transpose(
                                    r(pv[:, 64 * w : 64 * (w + 1)]),
                                    r(
                                        vpT[
                                            64 * pk : 64 * (pk + 1),
                                            ck // 2,
                                            256 * hl + 128 * (j % 2) : 256 * hl
                                            + 128 * (j % 2)
                                            + 128,
                                        ]
                                    ),
                                    r(
                                        ident[
                                            64 * pk : 64 * (pk + 1),
                                            64 * pk : 64 * (pk + 1),
                                        ]
                                    ),
                                )
                            nc.scalar.copy(
                                out=v_sb[:, hl, 8 * a : 8 * a + 8, :],
                                in_=pv.rearrange("p (j d) -> p j d", j=8),
                            )

                # ---------- attention for this group ----------
                with (
                    tc.tile_pool(name=f"mt{hg}", bufs=4) as mtp,
                    tc.tile_pool(name=f"sm{hg}", bufs=3) as smp,
                    tc.tile_pool(name=f"ep{hg}", bufs=4) as epool,
                    tc.tile_pool(name=f"ab{hg}", bufs=3) as abp,
                    tc.tile_pool(name=f"aT{hg}", bufs=2) as aTp,
                    tc.tile_pool(name=f"dn{hg}", bufs=6) as dnp,
                    tc.tile_pool(name=f"os{hg}", bufs=2) as ost,
                    tc.tile_pool(name=f"s{hg}", bufs=2, space="PSUM") as sps,
                    tc.tile_pool(name=f"t{hg}", bufs=2, space="PSUM") as tps,
                    tc.tile_pool(name=f"vt{hg}", bufs=2, space="PSUM") as vtp,
                ):
                    for cqg in range(2):
                        mts = []
                        for u2 in range(4):
                            cq = 4 * cqg + u2
                            mtile = mtp.tile([128, L], f32, tag="mt")
                            nc.gpsimd.dma_start(mtile[:], mb2[cq::8, :])
                            mts.append(mtile)
                        for hl in range(4):
                            h = 4 * hg + hl
                            aT = aTp.tile([128, 16, 512], bf16, tag="aT")
                            for u2 in range(4):
                                cq = 4 * cqg + u2
                                e_t = epool.tile([128, L], f32, tag="e")
                                sm_t = smp.tile([128, L], f32, tag="sm")
                                dent = dnp.tile([128, 4], f32, tag="den")
                                for pk in range(2):  # rhs partition half (ck parity)
                                    pss = sps.tile([128, 1024], f32, tag="s")
                                    # lhsT: parity-matched q projT slice
                                    if cq % 2 == pk:
                                        lhs = qpT[
                                            64 * pk : 64 * (pk + 1),
                                            cq // 2,
                                            128 * hl : 128 * (hl + 1),
                                        ]
                                    else:
                                        cq2 = (cq - 1) % 8
                                        lhs = qpTr[
                                            64 * pk : 64 * (pk + 1),
                                            cq2 // 2,
                                            128 * hl : 128 * (hl + 1),
                                        ]
                                    for i2 in range(2):
                                        rhs = kpT[
                                            64 * pk : 64 * (pk + 1),
                                            2 * i2 : 2 * (i2 + 1),
                                            256 * hl : 256 * (hl + 1),
                                        ]
                                        nc.tensor.matmul(
                                            pss[:, 512 * i2 : 512 * (i2 + 1)],
                                            lhs,
                                            rhs,
                                            start=True,
                                            stop=True,
                                        )
                                    # additive mask (blocked col order matches psum)
                                    nc.vector.tensor_tensor(
                                        out=sm_t[:, 1024 * pk : 1024 * (pk + 1)],
                                        in0=pss[:],
                                        in1=mts[u2][:, 1024 * pk : 1024 * (pk + 1)],
                                        op=add,
                                    )
                                    # exp: un-permute k order via strided out
                                    eview = e_t.rearrange("p (rk c) -> p c rk", c=8)[
                                        :, pk : 8 : 2, :
                                    ]
                                    nc.scalar.activation(
                                        eview,
                                        sm_t[
                                            :, 1024 * pk : 1024 * (pk + 1)
                                        ].rearrange("p (c rk) -> p c rk", rk=256),
                                        Exp,
                                        accum_out=dent[:, pk : pk + 1],
                                    )
                                nc.vector.tensor_tensor(
                                    out=dent[:, 2:3],
                                    in0=dent[:, 0:1],
                                    in1=dent[:, 1:2],
                                    op=add,
                                )
                                nc.vector.reciprocal(dent[:, 3:4], dent[:, 2:3])
                                nc.vector.tensor_scalar(
                                    out=e_t[:],
                                    in0=e_t[:],
                                    scalar1=dent[:, 3:4],
                                    scalar2=None,
                                    op0=mult,
                                )
                                nc.gpsimd.dma_start(attn_o[h, cq::8, :], e_t[:])
                                # bf16 copy of normalized attn for the @v path
                                a_bf = abp.tile([128, L], bf16, tag="ab")
                                if u2 == 0:
                                    nc.scalar.copy(out=a_bf[:], in_=e_t[:])
                                else:
                                    nc.vector.tensor_copy(out=a_bf[:], in_=e_t[:])
                                # transposes of blocked views for the @v matmul
                                ebl = a_bf.rearrange("p (rk c) -> p c rk", c=8)
                                for a in range(4):
                                    pt = tps.tile([128, 512], bf16, tag="tb512")
                                    for w in range(4):
                                        j = 4 * a + w
                                        ck = CKS[j // 2]
                                        nc.tensor.transpose(
                                            pt[:, 128 * w : 128 * (w + 1)],
                                            ebl[
                                                :,
                                                ck,
                                                128 * (j % 2) : 128 * (j % 2) + 128,
                                            ],
                                            identb_sb[:],
                                        )
                                    cpo = aT[
                                        :, 4 * a : 4 * a + 4, 128 * u2 : 128 * (u2 + 1)
                                    ]  # u2 in 0..4 -> cols 0..512
                                    cpi = pt.rearrange("p (j q) -> p j q", j=4)
                                    if a % 2 == 0:
                                        nc.scalar.copy(out=cpo, in_=cpi)
                                    else:
                                        nc.vector.tensor_copy(out=cpo, in_=cpi)
                            # attn @ v for (head, cq quad)
                            pvt = vtp.tile([DH, 512], f32, tag="pvt")
                            for j in range(16):
                                nc.tensor.matmul(
                                    pvt[:],
                                    v_sb[:, hl, j, :],
                                    aT[:, j, :],
                                    start=(j == 0),
                                    stop=(j == 15),
                                )
                            # outT[d', hl, 8*rq + 4*cqg + u2] <- pvt[d', 128u2+rq]
                            dstT = outT[:, hl, :].rearrange(
                                "p (rq c) -> p c rq", c=8
                            )[:, 4 * cqg : 4 * cqg + 4, :]
                            nc.scalar.copy(
                                out=r(dstT),
                                in_=pvt.rearrange("p (c rq) -> p c rq", rq=128),
                            )

                    # output projection for this group
                    for hl in range(4):
                        h = 4 * hg + hl
                        po_t = sps.tile([128, 1024], f32, tag="s")
                        po = po_t[:, 0:512]
                        for c in range(8):
                            nc.tensor.matmul(
                                po[:],
                                r(outT[:, hl, c::8]),
                                r(wo_sb[:, c, :]),
                                start=(c == 0),
                                stop=False,
                            )
                        nc.tensor.matmul(
                            po[:],
                            r(ones[0:1, 0:128]),
                            r(brow["bo"][0:1, :]),
                            start=False,
                            stop=True,
                        )
                        o_sb = ost.tile([128, D], f32, tag="ost")
                        nc.scalar.copy(out=o_sb[:], in_=po[:])
                        nc.gpsimd.dma_start(out_o[128 * h : 128 * (h + 1), :], o_sb[:])

    nc.finalize()
    return nc


def _get_nc():
    if "nc" not in _CACHE:
        _CACHE["nc"] = _build_nc()
    return _CACHE["nc"]


def _host_prep(inputs):
    Q = np.asarray(inputs["Q"], dtype=np.float32)
    K = np.asarray(inputs["K"], dtype=np.float32)
    V = np.asarray(inputs["V"], dtype=np.float32)
    mask = np.asarray(inputs["mask"])
    Wq = np.asarray(inputs["Wq"], dtype=np.float32) / 32.0
    bqv = np.asarray(inputs["bq"], dtype=np.float32) / 32.0
    Wqr = np.roll(Wq, -64, axis=1)
    bqr = np.roll(bqv, -64)
    com = {
        "wq": np.ascontiguousarray(Wq),
        "wqr": np.ascontiguousarray(Wqr),
        "wk": np.ascontiguousarray(np.asarray(inputs["Wk"], dtype=np.float32)),
        "wv": np.ascontiguousarray(np.asarray(inputs["Wv"], dtype=np.float32)),
        "wo": np.ascontiguousarray(np.asarray(inputs["Wo"], dtype=np.float32)),
        "bq": np.ascontiguousarray(bqv.reshape(1, D)),
        "bqr": np.ascontiguousarray(bqr.reshape(1, D)),
        "bk": np.ascontiguousarray(np.asarray(inputs["bk"], dtype=np.float32).reshape(1, D)),
        "bv": np.ascontiguousarray(np.asarray(inputs["bv"], dtype=np.float32).reshape(1, D)),
        "bo": np.ascontiguousarray(np.asarray(inputs["bo"], dtype=np.float32).reshape(1, D)),
        "onesr": np.ones((1, D), np.float32),
        "identr": np.eye(128, dtype=np.float32),
        "identb": np.eye(128, dtype=np.float32).astype(__import__("ml_dtypes").bfloat16),
    }
    qrows = np.concatenate([np.arange(256 * h, 256 * h + 128) for h in range(H)])
    in_maps = []
    for c in range(NCORES):
        b, g = c // 2, c % 2
        mslice = mask[b, 0, 1024 * g : 1024 * (g + 1), :].astype(np.float32)
        mbias = (mslice - 1.0) * 1e9
        # permute k columns to the blocked order the scores psum produces
        mb2v = mbias.reshape(LQ, 256, 8)[:, :, CKS].transpose(0, 2, 1).reshape(LQ, L)
        in_maps.append(
            {
                "xqt": np.ascontiguousarray(Q[b][qrows + 128 * g].T),
                "xkt": np.ascontiguousarray(K[b].T),
                "xvt": np.ascontiguousarray(V[b].T),
                "mb2": np.ascontiguousarray(mb2v),
                **com,
            }
        )
    return in_maps


def kernel(**inputs):
    from concourse.bass_utils import run_bass_kernel_spmd

    nc = _get_nc()
    in_maps = _host_prep(inputs)
    res = run_bass_kernel_spmd(nc, in_maps, list(range(NCORES))).results

    out = np.empty((B, L, D), np.float32)
    attn = np.empty((B, H, L, L), np.float32)
    for c in range(NCORES):
        b, g = c // 2, c % 2
        attn[b, :, 1024 * g : 1024 * (g + 1), :] = res[c]["attn"]
        o = res[c]["out"]
        for h in range(H):
            out[b, 256 * h + 128 * g : 256 * h + 128 * g + 128, :] = o[
                128 * h : 128 * (h + 1)
            ]
    return out, attn
